# revision 1
# baseline (speedup 1.0000x reference)
"""Entropy-regularized attention (standard MHA fwd) on 8 trn2 cores.

Sharding: core c -> batch b=c//4, head-group g=c%4 (4 of 16 heads).
Each core computes q/k/v for its 256-wide head-group slice, transposed-
layout attention (scores^T = K^T-stationary matmuls, exp on ACT, AV with
v-stationary producing avT), then a row-split Wo partial product.
Host sums the 4 partials per batch and adds bo (the "all-reduce").

v2 restructure vs baseline (306us -> ~252-258us):
- x / Wq / Wk / Wv / Wo shipped as bf16, host-prearranged into the
  exact SBUF layouts so every DMA is contiguous per partition; xT
  arrives in column blocks ordered to unblock the first projections.
- Attention loop is software-pipelined: scores+exp for tile i+1 are
  emitted before AV of tile i, so the PE never sits on the exp latency
  and the exp pipeline (ACT) stays fed.
- All fill work (v projections, mt1 q/k projections, Wo partials of
  the previous query-group, softmax-normalize) lives in a deferred
  queue popped one small closure per loop iteration, sized ~0.5-2us so
  the ACT-paced units absorb it without starving the exp stream.
- Softmax denominators of all heads are normalized with ONE Ln + ONE
  Exp batched on partitions 0/32/64/96 (ACT cost is free-size-driven,
  so per-head [1,1024] activations would waste 4x ACT time). The last
  head of the last query group runs its own short Ln/Exp chain read
  straight from PSUM so the post-loop tail is minimal; the other three
  heads of that group normalize while head 3 still computes.
- The upfront projection phase is one continuous PE stretch: the
  tensor clock needs ~3us of uninterrupted work to reach max p-state,
  so fragmenting it across DMA waits runs everything at ~60% clock.
- Output is written bf16; Wo-partial PSUM evictions in the tail use
  the then-idle ACT engine (copy lives in every act table set).
"""

import sys

for _p in ("/opt/trn_rl_repo", "/root/.axon_site/_ro/trn_rl_repo"):
    if _p not in sys.path:
        sys.path.insert(0, _p)

import numpy as np

import concourse.bass as bass
import concourse.mybir as mybir
import concourse.tile as tile
from concourse import bacc

P = 128
S = 2048  # sequence length
D = 1024  # hidden
DG = 256  # per-core head-group width (4 heads x 64)
HD = 64
NHL = 4  # heads per core
KT_D = D // P  # 8 contraction tiles for projections
ST = S // P  # 16 sequence tiles
QG = 1024  # qi group size (PSUM budget: scores 2x2 banks + av 2 + ops 2)
NQG = S // QG

F32 = mybir.dt.float32
F32R = mybir.dt.float32r
BF16 = mybir.dt.bfloat16


def build_nc():
    nc = bacc.Bacc(None, target_bir_lowering=False)

    # all inputs pre-arranged on the host into the exact SBUF layouts so
    # every DMA is a contiguous-per-partition blob (large descriptors)
    xT = nc.dram_tensor("xT", [P, 4 * KT_D * 512], BF16, kind="ExternalInput")
    wq = nc.dram_tensor("wq", [P, KT_D * DG], BF16, kind="ExternalInput")
    wk = nc.dram_tensor("wk", [P, KT_D * DG], BF16, kind="ExternalInput")
    wv = nc.dram_tensor("wv", [P, KT_D * DG], BF16, kind="ExternalInput")
    wo = nc.dram_tensor("wo", [P, 2 * D], BF16, kind="ExternalInput")
    bq = nc.dram_tensor("bq", [P, 2], F32, kind="ExternalInput")
    bk = nc.dram_tensor("bk", [P, 2], F32, kind="ExternalInput")
    bv = nc.dram_tensor("bv", [1, DG], F32R, kind="ExternalInput")
    out = nc.dram_tensor("out", [S, D], BF16, kind="ExternalOutput")

    with tile.TileContext(nc) as tc:
        _body(tc, nc, xT, wq, wk, wv, wo, bq, bk, bv, out)

    # Pin Exp/Ln to the one table set holding both: strip them from the
    # competing sets (dict order and size preserved, so act_func_set_id
    # indices stay valid). Without this the table-load pass alternates
    # exp_and_others <-> natural_log per head (~17 x 1.3us + PE stalls).
    import concourse.bacc as _bacc_mod

    _orig_tables = _bacc_mod.get_activation_tables

    def _pinned_tables(arch):
        t = _orig_tables(arch)
        for name, fns in t.items():
            if name != "natural_log_exp_and_others":
                fns.discard(mybir.ActivationFunctionType.Exp)
                fns.discard(mybir.ActivationFunctionType.Ln)
        return t

    _bacc_mod.get_activation_tables = _pinned_tables
    try:
        nc.compile()
    finally:
        _bacc_mod.get_activation_tables = _orig_tables
    return nc


def _body(tc, nc, xT, wq, wk, wv, wo, bq, bk, bv, out):
    from collections import deque
    from contextlib import ExitStack

    with ExitStack() as ctx:
        ctx.enter_context(
            nc.allow_low_precision(
                reason="bf16 matmul inputs; accumulation is fp32 PSUM"
            )
        )
        persist = ctx.enter_context(tc.tile_pool(name="persist", bufs=1))
        expool = ctx.enter_context(tc.tile_pool(name="expool", bufs=4))
        npool = ctx.enter_context(tc.tile_pool(name="npool", bufs=2))
        opool = ctx.enter_context(tc.tile_pool(name="opool", bufs=3))
        # PSUM budget (8 banks): sc 2x[128,1024]=4, av 1x[128,1024]=2,
        # ops 2x[128,512]=2. qkv/v/rb/proj tiles all use the ops slots.
        ps_sc = ctx.enter_context(tc.tile_pool(name="ps_sc", bufs=2, space="PSUM"))
        ps_av = ctx.enter_context(tc.tile_pool(name="ps_av", bufs=1, space="PSUM"))
        ps_o = ctx.enter_context(tc.tile_pool(name="ps_o", bufs=2, space="PSUM"))

        qT_sb = persist.tile([P, 2, S], BF16)
        kT_sb = persist.tile([P, 2, S], BF16)
        v_sb = persist.tile([P, ST, NHL * 65], BF16)  # 65-striped: col 64 = ones
        avT = [
            persist.tile([P, 2, QG], BF16, tag=f"avT{g}", name=f"avT{g}")
            for g in range(NQG)
        ]
        wo_sb = persist.tile([P, 2, D], BF16)
        ones_row = persist.tile([1, P], F32R)
        xT_sb = persist.tile([P, 4, KT_D, 512], BF16)
        wq_sb = persist.tile([P, KT_D, DG], BF16, tag="wq")
        wk_sb = persist.tile([P, KT_D, DG], BF16, tag="wk")
        wv_sb = persist.tile([P, KT_D, DG], BF16, tag="wv")
        bq_sb = persist.tile([P, 2], F32, tag="bq")
        bk_sb = persist.tile([P, 2], F32, tag="bk")
        bv_sb = persist.tile([1, DG], F32R, tag="bv")

        # DMAs ordered to match first-use: wk then the first xT column
        # block gate the first projection chain. All sources are
        # host-prearranged so each transfer is contiguous per partition.
        xTr = xT.rearrange("p (cb kt s) -> p cb kt s", kt=KT_D, s=512)
        nc.sync.dma_start(wk_sb[:], wk.rearrange("p (kt n) -> p kt n", n=DG))
        nc.sync.dma_start(xT_sb[:, 0, 0:4], xTr[:, 0, 0:4])
        nc.sync.dma_start(xT_sb[:, 0, 4:8], xTr[:, 0, 4:8])
        nc.sync.dma_start(bk_sb[:], bk[:])
        nc.sync.dma_start(bq_sb[:], bq[:])
        nc.sync.dma_start(wq_sb[:], wq.rearrange("p (kt n) -> p kt n", n=DG))
        nc.sync.dma_start(wv_sb[:], wv.rearrange("p (kt n) -> p kt n", n=DG))
        nc.sync.dma_start(bv_sb[:], bv[:])
        nc.sync.dma_start(xT_sb[:, 1], xTr[:, 1])
        nc.sync.dma_start(xT_sb[:, 2], xTr[:, 2])
        nc.sync.dma_start(xT_sb[:, 3], xTr[:, 3])
        nc.sync.dma_start(wo_sb[:], wo.rearrange("p (kt n) -> p kt n", n=D))

        # memset can't emit float32r; stage fp32 ones and copy-cast (rounds)
        ones_f32 = persist.tile([P, P], F32)
        nc.vector.memset(ones_f32[:], 1.0)
        nc.vector.tensor_copy(ones_row[:], ones_f32[0:1, :])
        ones_all = persist.tile([P, P], BF16)
        nc.vector.tensor_copy(ones_all[:], ones_f32[:])
        nc.vector.tensor_copy(
            v_sb.rearrange("p st (h w) -> p st h w", w=65)[:, :, :, 64],
            ones_f32[:, 0:64].rearrange("p (st h) -> p st h", h=4),
        )

        # ---- projection building blocks ----
        def qk_half(ps, wsb, mt, nq, kts):
            for kt in kts:
                nc.tensor.matmul(
                    ps[:, 0:512],
                    wsb[:, kt, mt * P : (mt + 1) * P],
                    xT_sb[:, nq, kt, :],
                    start=(kt == 0),
                    stop=(kt == KT_D - 1),
                )

        def qk_evict(ps, bsb, dest, mt, nq):
            nc.vector.tensor_scalar_add(
                dest[:, mt, nq * 512 : (nq + 1) * 512],
                ps[:, 0:512],
                bsb[:, mt : mt + 1],
            )

        def qk_full(wsb, bsb, dest, mt, nq):
            ps = ps_o.tile([P, 512], F32, tag="ops", name="qkps")
            qk_half(ps, wsb, mt, nq, range(0, KT_D))
            qk_evict(ps, bsb, dest, mt, nq)

        def qk_deferred(wsb, bsb, dest, mt, nq, pieces=2):
            # split the 8-matmul K-chain into `pieces` closures; the last
            # one carries the bias eviction
            box = []
            step = KT_D // pieces

            def make(pi):
                def run():
                    if pi == 0:
                        box.append(
                            ps_o.tile([P, 512], F32, tag="ops", name="qkps")
                        )
                    ps = box[-1]
                    qk_half(ps, wsb, mt, nq, range(pi * step, (pi + 1) * step))
                    if pi == pieces - 1:
                        qk_evict(ps, bsb, dest, mt, nq)
                        box.pop()

                return run

            return [make(pi) for pi in range(pieces)]

        def v_group(st):
            ps = ps_o.tile([P, 512], F32, tag="ops", name="vps")
            for kt in range(KT_D):
                nc.tensor.matmul(
                    ps[:, 0:DG],
                    xT_sb[:, st // 4, kt, (st % 4) * P : (st % 4 + 1) * P],
                    wv_sb[:, kt, :],
                    start=(kt == 0),
                    stop=False,
                )
            nc.tensor.matmul(
                ps[:, 0:DG],
                ones_row[0:1, 0:P],
                bv_sb[0:1, :],
                start=False,
                stop=True,
            )
            nc.vector.tensor_copy(
                v_sb.rearrange("p st (h w) -> p st h w", w=65)[:, st, :, 0:64],
                ps[:, 0:DG].rearrange("p (h w) -> p h w", w=64),
            )

        # Upfront (overlaps input DMA): everything head 0/1 of qg0 needs
        # except v st>=2. Kept as one continuous PE stretch — fragmenting
        # it across DMA waits keeps the PE at the mid p-state (the clock
        # needs ~3us of uninterrupted work to reach max).
        qk_full(wk_sb, bk_sb, kT_sb, 0, 0)
        qk_full(wq_sb, bq_sb, qT_sb, 0, 0)
        v_group(0)
        v_group(1)
        qk_full(wk_sb, bk_sb, kT_sb, 0, 1)
        qk_full(wq_sb, bq_sb, qT_sb, 0, 1)
        qk_full(wk_sb, bk_sb, kT_sb, 0, 2)
        qk_full(wk_sb, bk_sb, kT_sb, 0, 3)

        deferred = deque()
        for st in range(2, ST):  # units 0..13 (v st ready 2 units early)
            deferred.append(lambda st=st: v_group(st))
        for nq in range(4):  # units 14..21
            deferred.extend(qk_deferred(wk_sb, bk_sb, kT_sb, 1, nq))
        for nq in range(2):  # units 22..25
            deferred.extend(qk_deferred(wq_sb, bq_sb, qT_sb, 1, nq))
        for nq in range(2, 4):  # units 26..33 (quarter chains)
            deferred.extend(qk_deferred(wq_sb, bq_sb, qT_sb, 0, nq, pieces=4))
        for nq in range(2, 4):  # units 34..41
            deferred.extend(qk_deferred(wq_sb, bq_sb, qT_sb, 1, nq, pieces=4))

        # ---- attention phase (software-pipelined) ----
        def emit_sc_exp(qg, h, kt):
            mt, po = h // 2, (h % 2) * 64
            q0 = qg * QG
            sc = ps_sc.tile([P, QG], F32, tag="sc", name="sc")
            for nq in range(QG // 512):
                nc.tensor.matmul(
                    sc[:, nq * 512 : (nq + 1) * 512],
                    kT_sb[po : po + 64, mt, kt * P : (kt + 1) * P],
                    qT_sb[
                        po : po + 64,
                        mt,
                        q0 + nq * 512 : q0 + (nq + 1) * 512,
                    ],
                    start=True,
                    stop=True,
                )
            ex = expool.tile([P, QG], BF16, tag="ex", name="ex")
            nc.scalar.activation(
                ex[:], sc[:], mybir.ActivationFunctionType.Exp, scale=0.125
            )
            return ex

        uls = {}
        l4s = {}
        rs = {}

        def finish_head(qg, h, av):
            # batched norm: head h's l goes to partition 32h so the rb
            # broadcast matmul gets a legal base partition (0/32/64).
            # Gather l first (small) so the norm's Ln isn't gated on the
            # big ul eviction. The very last head instead runs its own
            # Ln/Exp with the denominator read straight from PSUM, so the
            # tail chain after the final exp is as short as possible.
            last_head = qg == NQG - 1 and h == NHL - 1
            if h == 0:
                l4s[qg] = npool.tile([97, QG], F32, tag="l4", name="l4")
                nc.gpsimd.memset(l4s[qg][:], 1.0)
            if not last_head:
                nc.vector.tensor_copy(
                    l4s[qg][32 * h : 32 * h + 1, :], av[64:65, :]
                )
            else:
                ln1 = npool.tile([1, QG], F32, tag="ln1", name="ln1")
                nc.scalar.activation(
                    ln1[:], av[64:65, :], mybir.ActivationFunctionType.Ln
                )
                r1 = npool.tile([1, QG], BF16, tag="r1", name="r1")
                nc.scalar.activation(
                    r1[:], ln1[:], mybir.ActivationFunctionType.Exp, scale=-1.0
                )
                rs[(qg, h)] = (r1, 0)
            ul = npool.tile([65, QG], F32R, tag="ul", bufs=4, name="ul")
            nc.vector.tensor_copy(ul[:], av[0:65, :])
            uls[(qg, h)] = ul

        def rb_mult_nq(qg, h, nq):
            mt, po = h // 2, (h % 2) * 64
            ul = uls[(qg, h)] if nq == 0 else uls.pop((qg, h))
            r_ap, base = rs[(qg, h)]
            rb = ps_o.tile([P, 512], F32, tag="ops", name="rb")
            nc.tensor.matmul(
                rb[0:64, :],
                ones_all[base : base + 1, 0:64],
                r_ap[base : base + 1, nq * 512 : (nq + 1) * 512],
                start=True,
                stop=True,
            )
            nc.vector.tensor_mul(
                out=avT[qg][po : po + 64, mt, nq * 512 : (nq + 1) * 512],
                in0=ul[0:64, nq * 512 : (nq + 1) * 512],
                in1=rb[0:64, :],
            )

        ot_box = {}

        def phase_d_nd(qg, sti, nd):
            st = qg * (QG // P) + sti
            last = qg == NQG - 1
            if nd == 0:
                ot_box[(qg, sti)] = opool.tile(
                    [P, D], BF16, tag="ot", name="ot"
                )
            ot = ot_box[(qg, sti)]
            pp = ps_o.tile([P, 512], F32, tag="ops", name="pp")
            for kt2 in range(2):
                nc.tensor.matmul(
                    pp[:],
                    avT[qg][:, kt2, sti * P : (sti + 1) * P],
                    wo_sb[:, kt2, nd * 512 : (nd + 1) * 512],
                    start=(kt2 == 0),
                    stop=(kt2 == 1),
                )
            if last and nd == 0:
                # ACT is idle in the tail (all exps done): use it for half
                # the PSUM evictions (copy is in every act table set, so
                # no table switch)
                nc.scalar.activation(
                    ot[:, nd * 512 : (nd + 1) * 512],
                    pp[:],
                    mybir.ActivationFunctionType.Copy,
                )
            else:
                nc.vector.tensor_copy(ot[:, nd * 512 : (nd + 1) * 512], pp[:])
            if nd == 1:
                del ot_box[(qg, sti)]
                nc.sync.dma_start(out[st * P : (st + 1) * P, :], ot[:])

        def make_norm(qg, heads, items=()):
            def fire():
                # one Ln + one Exp for all heads' denominators at once:
                # 1/l = exp(-ln(l)); ACT cost is free-size-driven so the
                # [97,1024] batch costs the same as a single [1,1024].
                hi = 32 * max(heads) + 1
                ls = npool.tile([97, QG], F32, tag="ls", name="ls")
                nc.scalar.activation(
                    ls[0:hi], l4s[qg][0:hi],
                    mybir.ActivationFunctionType.Ln,
                )
                r4 = npool.tile([97, QG], BF16, tag="r4", name="r4")
                nc.scalar.activation(
                    r4[0:hi], ls[0:hi],
                    mybir.ActivationFunctionType.Exp, scale=-1.0,
                )
                for h in heads:
                    if h == 3:
                        # matmul base partitions may only be 0/32/64
                        r3 = npool.tile([1, QG], BF16, tag="r3", name="r3")
                        nc.vector.tensor_copy(r3[:], r4[96:97, :])
                        rs[(qg, h)] = (r3, 0)
                    else:
                        rs[(qg, h)] = (r4, 32 * h)
                deferred.extend(items)

            return fire

        units = [
            (qg, h, kt)
            for qg in range(NQG)
            for h in range(NHL)
            for kt in range(ST)
        ]
        ex_tiles = {0: emit_sc_exp(*units[0])}
        av = None
        pending_norm = None
        for i, (qg, h, kt) in enumerate(units):
            if i + 1 < len(units):
                ex_tiles[i + 1] = emit_sc_exp(*units[i + 1])
            if pending_norm is not None and kt in (1, 3):
                pending_norm()
                pending_norm = None
            if deferred:
                deferred.popleft()()
            if kt == 0:
                av = ps_av.tile([P, QG], F32, tag="av", name="av")
            ex = ex_tiles.pop(i)
            for nq in range(QG // 512):
                nc.tensor.matmul(
                    av[0:65, nq * 512 : (nq + 1) * 512],
                    v_sb[:, kt, h * 65 : h * 65 + 65],
                    ex[:, nq * 512 : (nq + 1) * 512],
                    start=(kt == 0),
                    stop=(kt == ST - 1),
                )
            if kt == ST - 1:
                finish_head(qg, h, av)
                if h == NHL - 1 and qg < NQG - 1:
                    # full 4-head norm; fires at kt==1 of the next head
                    items = []
                    for nq in range(QG // 512):
                        for hh in range(NHL):
                            items.append(
                                lambda hh=hh, nq=nq, qg=qg:
                                rb_mult_nq(qg, hh, nq)
                            )
                        for sti in range(nq * 4, nq * 4 + 4):
                            for nd in range(2):
                                items.append(
                                    lambda sti=sti, nd=nd, qg=qg:
                                    phase_d_nd(qg, sti, nd)
                                )
                    pending_norm = make_norm(qg, (0, 1, 2, 3), items)
                elif h == NHL - 2 and qg == NQG - 1:
                    # last qg: normalize h0..h2 while h3 still computes so
                    # the tail only waits on h3's own norm chain. Fires at
                    # kt==1 (after h2's denominator gather lands).
                    items = []
                    for nq in range(QG // 512):
                        for hh in range(NHL - 1):
                            items.append(
                                lambda hh=hh, nq=nq, qg=qg:
                                rb_mult_nq(qg, hh, nq)
                            )
                    pending_norm = make_norm(qg, (0, 1, 2), items)

        # tail: h3's rb/mult interleaved with the Wo partials
        qg = NQG - 1
        for nq in range(QG // 512):
            rb_mult_nq(qg, NHL - 1, nq)
            for sti in range(nq * 4, nq * 4 + 4):
                phase_d_nd(qg, sti, 0)
                phase_d_nd(qg, sti, 1)
        while deferred:
            deferred.popleft()()


_NC_CACHE = None


def get_nc():
    global _NC_CACHE
    if _NC_CACHE is None:
        _NC_CACHE = build_nc()
    return _NC_CACHE


def make_in_maps(x, Wq, bq, Wk, bk, Wv, bv, Wo, bo):
    import ml_dtypes

    bf16 = ml_dtypes.bfloat16

    def w_arr(W, sl):
        # [D, DG] -> [p, kt*DG]: W[kt*128+p, n] at [p, kt, n]
        return np.ascontiguousarray(
            W[:, sl].reshape(KT_D, P, DG).transpose(1, 0, 2).reshape(P, -1)
        ).astype(bf16)

    in_maps = []
    for c in range(8):
        b, g = c // 4, c % 4
        sl = slice(g * DG, (g + 1) * DG)
        # x[b].T is [D, S]; SBUF wants [p, cb, kt, 512] with row kt*128+p,
        # col cb*512+s
        xt = (
            x[b]
            .T.reshape(KT_D, P, 4, 512)
            .transpose(1, 2, 0, 3)
            .reshape(P, -1)
            .astype(bf16)
        )
        wo_a = np.ascontiguousarray(
            Wo[sl, :].reshape(2, P, D).transpose(1, 0, 2).reshape(P, -1)
        ).astype(bf16)
        in_maps.append(
            {
                "xT": np.ascontiguousarray(xt),
                "wq": w_arr(Wq, sl),
                "wk": w_arr(Wk, sl),
                "wv": w_arr(Wv, sl),
                "wo": wo_a,
                "bq": np.ascontiguousarray(bq[sl].reshape(2, P).T),
                "bk": np.ascontiguousarray(bk[sl].reshape(2, P).T),
                "bv": np.ascontiguousarray(bv[sl].reshape(1, DG)),
            }
        )
    return in_maps


def kernel(x, Wq, bq, Wk, bk, Wv, bv, Wo, bo, _run_kwargs=None):
    from concourse.bass_utils import run_bass_kernel_spmd

    x = np.asarray(x, dtype=np.float32)
    nc = get_nc()
    in_maps = make_in_maps(
        x,
        np.asarray(Wq, np.float32),
        np.asarray(bq, np.float32),
        np.asarray(Wk, np.float32),
        np.asarray(bk, np.float32),
        np.asarray(Wv, np.float32),
        np.asarray(bv, np.float32),
        np.asarray(Wo, np.float32),
        np.asarray(bo, np.float32),
    )
    res = run_bass_kernel_spmd(
        nc, in_maps, core_ids=list(range(8)), **(_run_kwargs or {})
    )
    bo = np.asarray(bo, np.float32)
    outp = np.empty((2, S, D), dtype=np.float32)
    for b in range(2):
        acc = res.results[4 * b]["out"].astype(np.float32)
        for g in range(1, 4):
            acc = acc + res.results[4 * b + g]["out"].astype(np.float32)
        outp[b] = acc + bo[None, :]
    kernel.last_result = res
    return outp



# revision 10
# speedup vs baseline: 1.0164x; 1.0164x over previous
"""Entropy-regularized attention (standard MHA fwd) on 8 trn2 cores.

Sharding: core c -> batch b=c//4, head-group g=c%4 (4 of 16 heads).
Each core computes q/k/v for its 256-wide head-group slice, transposed-
layout attention, then a row-split Wo partial product. Host sums the 4
partials per batch and adds bo (the "all-reduce").

v3 restructure vs v2 (261us -> target ~160us):
- Scores are emitted as ROW-TILED HEAD PAIRS: the two heads of an mt
  group have K=64 stationaries at base partitions 0 and 64, so their
  score matmuls auto-derive tile_position (0,0)/(64,0) and execute
  CONCURRENTLY on the PE (each K=64 tile uses half the array rows).
  This halves the PE time of the scores phase.
- QG=512 with 4 query groups: per pair-unit one [128,2,512] PSUM tile
  holds both heads' scores and ONE Exp activation (N=1024) covers the
  pair, keeping ACT's per-instruction overhead amortized while PSUM
  stays within 8 banks (sc 2x2 + av 2 + ops 2).
- Softmax normalization no longer uses ACT Ln/Exp nor PE broadcast
  matmuls: DVE reciprocal computes 1/l, GPSIMD partition_broadcast
  fans it across 64 partitions, DVE tensor_mul normalizes into avT.
  ACT does nothing but the exp stream; the PE does nothing but matmul.
- ALL non-score PE work (q/k/v projections, av accumulation, Wo
  partials) lives in deferred queues popped under a per-unit cost
  budget (~880ns) between score emissions, so the exp stream is never
  blocked by more than one unit's worth of filler. av matmuls are
  gated on their v-projection pieces to preserve FIFO progress.
- Unit order: pair 01 for all 4 qgs (kt-inner), then pair 23. kT/qT
  mt=1 projections amortize over the first 64 units; Wo partials for
  each qg unlock after its second pair's norms and fill the late
  units, leaving only qg3's Wo (+ final norm chain) in the tail.
"""

import sys

for _p in ("/opt/trn_rl_repo", "/root/.axon_site/_ro/trn_rl_repo"):
    if _p not in sys.path:
        sys.path.insert(0, _p)

import numpy as np

import concourse.bass as bass
import concourse.mybir as mybir
import concourse.tile as tile
from concourse import bacc

P = 128
S = 2048  # sequence length
D = 1024  # hidden
DG = 256  # per-core head-group width (4 heads x 64)
HD = 64
NHL = 4  # heads per core
KT_D = D // P  # 8 contraction tiles for projections
ST = S // P  # 16 sequence tiles
QG = 512  # query-group width
NQG = S // QG  # 4 query groups
NPR = 2  # head pairs per core

F32 = mybir.dt.float32
F32R = mybir.dt.float32r
BF16 = mybir.dt.bfloat16


def build_nc():
    nc = bacc.Bacc(None, target_bir_lowering=False)

    # all inputs pre-arranged on the host into the exact SBUF layouts so
    # every DMA is a contiguous-per-partition blob (large descriptors)
    xT = nc.dram_tensor("xT", [P, 4 * KT_D * 512], BF16, kind="ExternalInput")
    wq = nc.dram_tensor("wq", [P, KT_D * DG], BF16, kind="ExternalInput")
    wk = nc.dram_tensor("wk", [P, KT_D * DG], BF16, kind="ExternalInput")
    wv = nc.dram_tensor("wv", [P, KT_D * DG], BF16, kind="ExternalInput")
    wo = nc.dram_tensor("wo", [P, 2 * D], BF16, kind="ExternalInput")
    bq = nc.dram_tensor("bq", [P, 2], F32, kind="ExternalInput")
    bk = nc.dram_tensor("bk", [P, 2], F32, kind="ExternalInput")
    bv = nc.dram_tensor("bv", [1, DG], F32R, kind="ExternalInput")
    out = nc.dram_tensor("out", [S, D], BF16, kind="ExternalOutput")

    with tile.TileContext(nc) as tc:
        _body(tc, nc, xT, wq, wk, wv, wo, bq, bk, bv, out)

    # Pin Exp/Ln to the one table set holding both: strip them from the
    # competing sets (dict order and size preserved, so act_func_set_id
    # indices stay valid). Without this the table-load pass can bounce
    # between table sets.
    import concourse.bacc as _bacc_mod

    _orig_tables = _bacc_mod.get_activation_tables

    def _pinned_tables(arch):
        t = _orig_tables(arch)
        for name, fns in t.items():
            if name != "natural_log_exp_and_others":
                fns.discard(mybir.ActivationFunctionType.Exp)
                fns.discard(mybir.ActivationFunctionType.Ln)
        return t

    _bacc_mod.get_activation_tables = _pinned_tables
    try:
        nc.compile()
    finally:
        _bacc_mod.get_activation_tables = _orig_tables
    return nc


def _body(tc, nc, xT, wq, wk, wv, wo, bq, bk, bv, out):
    from collections import deque
    from contextlib import ExitStack

    with ExitStack() as ctx:
        ctx.enter_context(
            nc.allow_low_precision(
                reason="bf16 matmul inputs; accumulation is fp32 PSUM"
            )
        )
        persist = ctx.enter_context(tc.tile_pool(name="persist", bufs=1))
        expool = ctx.enter_context(tc.tile_pool(name="expool", bufs=16))
        ulpool = ctx.enter_context(tc.tile_pool(name="ulpool", bufs=4))
        rpool = ctx.enter_context(tc.tile_pool(name="rpool", bufs=4))
        opool = ctx.enter_context(tc.tile_pool(name="opool", bufs=3))
        # PSUM budget (8 banks): sc 2x[128,2,512]=4, av 2x[65,512]=2,
        # ops 2x[128,512]=2. qkv/v/proj/wo tiles all use the ops slots.
        ps_sc = ctx.enter_context(tc.tile_pool(name="ps_sc", bufs=2, space="PSUM"))
        ps_av = ctx.enter_context(tc.tile_pool(name="ps_av", bufs=2, space="PSUM"))
        ps_o = ctx.enter_context(tc.tile_pool(name="ps_o", bufs=2, space="PSUM"))

        qT_sb = persist.tile([P, 2, S], BF16)
        kT_sb = persist.tile([P, 2, S], BF16)
        v_sb = persist.tile([P, ST, NHL * 65], BF16)  # 65-striped: col 64 = ones
        avT = [
            persist.tile([P, 2, QG], BF16, tag=f"avT{g}", name=f"avT{g}")
            for g in range(NQG)
        ]
        wo_sb = persist.tile([P, 2, D], BF16)
        ones_row = persist.tile([1, P], F32R)
        xT_sb = persist.tile([P, 4, KT_D, 512], BF16)
        wq_sb = persist.tile([P, KT_D, DG], BF16, tag="wq")
        wk_sb = persist.tile([P, KT_D, DG], BF16, tag="wk")
        wv_sb = persist.tile([P, KT_D, DG], BF16, tag="wv")
        bq_sb = persist.tile([P, 2], F32, tag="bq")
        bk_sb = persist.tile([P, 2], F32, tag="bk")
        bv_sb = persist.tile([1, DG], F32R, tag="bv")

        # DMAs ordered to match first-use: wk then the first xT column
        # block gate the first projection chain.
        xTr = xT.rearrange("p (cb kt s) -> p cb kt s", kt=KT_D, s=512)
        nc.sync.dma_start(wk_sb[:], wk.rearrange("p (kt n) -> p kt n", n=DG))
        nc.sync.dma_start(xT_sb[:, 0, 0:4], xTr[:, 0, 0:4])
        nc.sync.dma_start(xT_sb[:, 0, 4:8], xTr[:, 0, 4:8])
        nc.sync.dma_start(bk_sb[:], bk[:])
        nc.sync.dma_start(bq_sb[:], bq[:])
        nc.sync.dma_start(wq_sb[:], wq.rearrange("p (kt n) -> p kt n", n=DG))
        nc.sync.dma_start(wv_sb[:], wv.rearrange("p (kt n) -> p kt n", n=DG))
        nc.sync.dma_start(bv_sb[:], bv[:])
        nc.sync.dma_start(xT_sb[:, 1], xTr[:, 1])
        nc.sync.dma_start(xT_sb[:, 2], xTr[:, 2])
        nc.sync.dma_start(xT_sb[:, 3], xTr[:, 3])
        nc.sync.dma_start(wo_sb[:], wo.rearrange("p (kt n) -> p kt n", n=D))

        # memset can't emit float32r; stage fp32 ones and copy-cast
        ones_f32 = persist.tile([P, P], F32)
        nc.vector.memset(ones_f32[:], 1.0)
        nc.vector.tensor_copy(ones_row[:], ones_f32[0:1, :])
        nc.vector.tensor_copy(
            v_sb.rearrange("p st (h w) -> p st h w", w=65)[:, :, :, 64],
            ones_f32[:, 0:64].rearrange("p (st h) -> p st h", h=4),
        )

        # ---- projection building blocks ----
        def qk_half(ps, wsb, mt, nq, kts):
            for kt in kts:
                nc.tensor.matmul(
                    ps[:, 0:512],
                    wsb[:, kt, mt * P : (mt + 1) * P],
                    xT_sb[:, nq, kt, :],
                    start=(kt == 0),
                    stop=(kt == KT_D - 1),
                )

        def qk_evict(ps, bsb, dest, mt, nq):
            nc.vector.tensor_scalar_add(
                dest[:, mt, nq * 512 : (nq + 1) * 512],
                ps[:, 0:512],
                bsb[:, mt : mt + 1],
            )

        # emission-order readiness flags: a score matmul may only be
        # EMITTED after the projection chain writing its qT/kT region has
        # been emitted (the Tile framework tracks writer->reader deps in
        # program order; a later-emitted writer would be a race).
        qk_ready = {}  # ("q"/"k", mt, nq) -> True

        def qk_full(wsb, bsb, dest, mt, nq, key):
            ps = ps_o.tile([P, 512], F32, tag="ops", name="qkps")
            qk_half(ps, wsb, mt, nq, range(0, KT_D))
            qk_evict(ps, bsb, dest, mt, nq)
            qk_ready[(key, mt, nq)] = True

        def qk_pieces(wsb, bsb, dest, mt, nq, key, pieces=4):
            # split the 8-matmul K-chain into `pieces` closures; the last
            # one carries the bias eviction. cost ~= (8/pieces)*213ns
            box = []
            step = KT_D // pieces

            def make(pi):
                def run():
                    if pi == 0:
                        box.append(
                            ps_o.tile([P, 512], F32, tag="ops", name="qkps")
                        )
                    ps = box[-1]
                    qk_half(ps, wsb, mt, nq, range(pi * step, (pi + 1) * step))
                    if pi == pieces - 1:
                        qk_evict(ps, bsb, dest, mt, nq)
                        box.pop()
                        qk_ready[(key, mt, nq)] = True

                return run

            return [(step * 215, make(pi)) for pi in range(pieces)]

        v_done = [False] * ST

        def v_group_half(st, half):
            # half 0: kts 0-3; half 1: kts 4-7 + bias + evict
            def run():
                if half == 0:
                    v_group_half.box[st] = ps_o.tile(
                        [P, 512], F32, tag="ops", name="vps"
                    )
                    ps = v_group_half.box[st]
                    for kt in range(0, 4):
                        nc.tensor.matmul(
                            ps[:, 0:DG],
                            xT_sb[:, st // 4, kt, (st % 4) * P : (st % 4 + 1) * P],
                            wv_sb[:, kt, :],
                            start=(kt == 0),
                            stop=False,
                        )
                else:
                    ps = v_group_half.box.pop(st)
                    for kt in range(4, KT_D):
                        nc.tensor.matmul(
                            ps[:, 0:DG],
                            xT_sb[:, st // 4, kt, (st % 4) * P : (st % 4 + 1) * P],
                            wv_sb[:, kt, :],
                            start=False,
                            stop=False,
                        )
                    nc.tensor.matmul(
                        ps[:, 0:DG],
                        ones_row[0:1, 0:P],
                        bv_sb[0:1, :],
                        start=False,
                        stop=True,
                    )
                    nc.vector.tensor_copy(
                        v_sb.rearrange("p st (h w) -> p st h w", w=65)[
                            :, st, :, 0:64
                        ],
                        ps[:, 0:DG].rearrange("p (h w) -> p h w", w=64),
                    )
                    v_done[st] = True

            return run

        v_group_half.box = {}

        def v_group_full(st):
            v_group_half(st, 0)()
            v_group_half(st, 1)()

        # ---- deferred machinery ----
        # avq: essential attention follow-up (av accumulation + norms),
        # popped first. defq: projection / Wo filler under a budget.
        avq = deque()
        defq = deque()
        pending_av = deque()  # av pieces waiting on their v group

        BUDGET = 880
        n_av_emitted = [0]  # count of av matmul pieces emitted (2 per unit)

        def drain_pending():
            while pending_av and v_done[pending_av[0][0]]:
                _, kind, cost, fn = pending_av.popleft()
                avq.append((kind, cost, fn))

        def pop_one_av():
            drain_pending()
            if avq:
                kind, _, fn = avq.popleft()
                fn()
                if kind == "av":
                    n_av_emitted[0] += 1
                return True
            return False

        def pop_one_def():
            if defq:
                _, fn = defq.popleft()
                fn()
                return True
            return False

        def pop_pieces():
            spent = 0
            drain_pending()
            while avq and spent < BUDGET:
                kind, cost, fn = avq.popleft()
                fn()
                if kind == "av":
                    n_av_emitted[0] += 1
                spent += cost
            while defq and spent < BUDGET:
                cost, fn = defq.popleft()
                fn()
                spent += cost

        def ensure_qk(key, mt, nq):
            # force-pop until the projection chain for this region has run
            while not qk_ready.get((key, mt, nq)):
                if not pop_one_def():
                    raise RuntimeError(f"deadlock: {key} mt{mt} nq{nq}")

        def ensure_av_through(unit_idx):
            # all av pieces of units <= unit_idx emitted (ex-slot reuse)
            while n_av_emitted[0] < 2 * (unit_idx + 1):
                if not pop_one_av() and not pop_one_def():
                    raise RuntimeError("deadlock: av drain")

        # ---- attention phase ----
        EXBUFS = 16  # must match expool bufs

        def sc_exp(pr, qg, kt):
            ensure_qk("k", pr, kt // 4)
            ensure_qk("q", pr, qg)
            mt, q0 = pr, qg * QG
            sc = ps_sc.tile([P, 2, 512], F32, tag="sc", name="sc")
            for hh in range(2):
                po = hh * 64
                nc.tensor.matmul(
                    sc[:, hh, :],
                    kT_sb[po : po + 64, mt, kt * P : (kt + 1) * P],
                    qT_sb[po : po + 64, mt, q0 : q0 + 512],
                    start=True,
                    stop=True,
                )
            ex = expool.tile([P, 2, 512], BF16, tag="ex", name="ex")
            nc.scalar.activation(
                ex[:], sc[:], mybir.ActivationFunctionType.Exp, scale=0.125
            )
            return ex

        avps = {}  # hh -> live av psum tile for current (pr, qg)

        def av_piece(pr, qg, kt, hh, ex):
            h = 2 * pr + hh

            def run():
                if kt == 0:
                    avps[hh] = ps_av.tile([65, 512], F32, tag="av", name="av")
                nc.tensor.matmul(
                    avps[hh][0:65, :],
                    v_sb[:, kt, h * 65 : h * 65 + 65],
                    ex[:, hh, :],
                    start=(kt == 0),
                    stop=(kt == ST - 1),
                )

            return (215, run)

        def norm_pieces(pr, qg, hh, last):
            mt, po = pr, hh * 64
            box = []

            def p1():
                av = avps.pop(hh)
                ul = ulpool.tile([65, 512], F32, tag="ul", name="ul")
                nc.vector.tensor_copy(ul[:], av[0:65, :])
                r = rpool.tile([1, 512], F32, tag="r", name="r")
                nc.vector.reciprocal(r[:], ul[64:65, :])
                box.append((ul, r))

            def p2():
                ul, r = box.pop()
                rbb = rpool.tile([64, 512], F32, tag="rbb", name="rbb")
                nc.gpsimd.partition_broadcast(rbb[:], r[:], channels=64)
                nc.vector.tensor_mul(
                    out=avT[qg][po : po + 64, mt, :],
                    in0=ul[0:64, :],
                    in1=rbb[:],
                )
                if last:
                    # second pair of this qg normalized -> Wo can run
                    defq.extend(wo_pieces(qg))

            return [(50, p1), (50, p2)]

        ot_box = {}

        def wo_pieces(qg):
            # per (sti, nd): 2 matmuls (kt2 accumulation) + eviction; the
            # ops-pool slot is alloc'd and freed within one piece so the
            # bufs=2 rotation can interleave with qk/v pieces.
            pieces = []

            def make(sti, nd):
                def run():
                    st = qg * (QG // P) + sti
                    if nd == 0:
                        ot_box[sti] = opool.tile([P, D], BF16, tag="ot", name="ot")
                    ot = ot_box[sti]
                    pp = ps_o.tile([P, 512], F32, tag="ops", name="pp")
                    for kt2 in range(2):
                        nc.tensor.matmul(
                            pp[:],
                            avT[qg][:, kt2, sti * P : (sti + 1) * P],
                            wo_sb[:, kt2, nd * 512 : (nd + 1) * 512],
                            start=(kt2 == 0),
                            stop=(kt2 == 1),
                        )
                    nc.vector.tensor_copy(ot[:, nd * 512 : (nd + 1) * 512], pp[:])
                    if nd == 1:
                        del ot_box[sti]
                        nc.sync.dma_start(out[st * P : (st + 1) * P, :], ot[:])

                return run

            for sti in range(QG // P):
                for nd in range(2):
                    pieces.append((460, make(sti, nd)))
            return pieces

        # ---- upfront phase (continuous PE stretch; overlaps input DMA) ----
        qk_full(wk_sb, bk_sb, kT_sb, 0, 0, "k")
        qk_full(wq_sb, bq_sb, qT_sb, 0, 0, "q")
        v_group_full(0)
        v_group_full(1)

        # ---- static filler: ordered by need-by unit ----
        # kT mt0 fully by unit ~12 (sc consumes kt blocks 4/8/12 at units
        # 4/8/12); q(mt0,nq1..3) by units 16/32/48; v(st) before av(st)
        # emission (forced by ensure_av_through); mt1 chains by unit 64.
        defq.extend(qk_pieces(wk_sb, bk_sb, kT_sb, 0, 1, "k"))
        defq.extend(qk_pieces(wk_sb, bk_sb, kT_sb, 0, 2, "k"))
        defq.extend(qk_pieces(wk_sb, bk_sb, kT_sb, 0, 3, "k"))
        defq.extend(qk_pieces(wq_sb, bq_sb, qT_sb, 0, 1, "q"))
        for st in (2, 3, 4, 5):
            defq.append((500, v_group_half(st, 0)))
            defq.append((560, v_group_half(st, 1)))
        defq.extend(qk_pieces(wq_sb, bq_sb, qT_sb, 0, 2, "q"))
        for st in (6, 7, 8, 9):
            defq.append((500, v_group_half(st, 0)))
            defq.append((560, v_group_half(st, 1)))
        defq.extend(qk_pieces(wq_sb, bq_sb, qT_sb, 0, 3, "q"))
        for st in (10, 11, 12, 13, 14, 15):
            defq.append((500, v_group_half(st, 0)))
            defq.append((560, v_group_half(st, 1)))
        for nq in range(4):
            defq.extend(qk_pieces(wk_sb, bk_sb, kT_sb, 1, nq, "k"))
            defq.extend(qk_pieces(wq_sb, bq_sb, qT_sb, 1, nq, "q"))

        # ---- the unit loop ----
        units = [
            (pr, qg, kt)
            for pr in range(NPR)
            for qg in range(NQG)
            for kt in range(ST)
        ]
        ex_tiles = {0: sc_exp(*units[0])}
        for i, (pr, qg, kt) in enumerate(units):
            if i + 1 < len(units):
                ensure_av_through(i + 1 - EXBUFS)
                ex_tiles[i + 1] = sc_exp(*units[i + 1])
            ex = ex_tiles.pop(i)
            for hh in range(2):
                pending_av.append((kt, "av", *av_piece(pr, qg, kt, hh, ex)))
            if kt == ST - 1:
                for hh in range(2):
                    for piece in norm_pieces(pr, qg, hh, pr == NPR - 1 and hh == 1):
                        pending_av.append((kt, "norm", *piece))
            pop_pieces()
        # ---- tail: drain everything ----
        while pending_av or avq or defq:
            if not pop_one_av():
                if not pop_one_def():
                    raise RuntimeError("tail deadlock")


_NC_CACHE = None


def get_nc():
    global _NC_CACHE
    if _NC_CACHE is None:
        _NC_CACHE = build_nc()
    return _NC_CACHE


def make_in_maps(x, Wq, bq, Wk, bk, Wv, bv, Wo, bo):
    import ml_dtypes

    bf16 = ml_dtypes.bfloat16

    def w_arr(W, sl):
        # [D, DG] -> [p, kt*DG]: W[kt*128+p, n] at [p, kt, n]
        return np.ascontiguousarray(
            W[:, sl].reshape(KT_D, P, DG).transpose(1, 0, 2).reshape(P, -1)
        ).astype(bf16)

    in_maps = []
    for c in range(8):
        b, g = c // 4, c % 4
        sl = slice(g * DG, (g + 1) * DG)
        # x[b].T is [D, S]; SBUF wants [p, cb, kt, 512] with row kt*128+p,
        # col cb*512+s
        xt = (
            x[b]
            .T.reshape(KT_D, P, 4, 512)
            .transpose(1, 2, 0, 3)
            .reshape(P, -1)
            .astype(bf16)
        )
        wo_a = np.ascontiguousarray(
            Wo[sl, :].reshape(2, P, D).transpose(1, 0, 2).reshape(P, -1)
        ).astype(bf16)
        in_maps.append(
            {
                "xT": np.ascontiguousarray(xt),
                "wq": w_arr(Wq, sl),
                "wk": w_arr(Wk, sl),
                "wv": w_arr(Wv, sl),
                "wo": wo_a,
                "bq": np.ascontiguousarray(bq[sl].reshape(2, P).T),
                "bk": np.ascontiguousarray(bk[sl].reshape(2, P).T),
                "bv": np.ascontiguousarray(bv[sl].reshape(1, DG)),
            }
        )
    return in_maps


def kernel(x, Wq, bq, Wk, bk, Wv, bv, Wo, bo, _run_kwargs=None):
    from concourse.bass_utils import run_bass_kernel_spmd

    x = np.asarray(x, dtype=np.float32)
    nc = get_nc()
    in_maps = make_in_maps(
        x,
        np.asarray(Wq, np.float32),
        np.asarray(bq, np.float32),
        np.asarray(Wk, np.float32),
        np.asarray(bk, np.float32),
        np.asarray(Wv, np.float32),
        np.asarray(bv, np.float32),
        np.asarray(Wo, np.float32),
        np.asarray(bo, np.float32),
    )
    res = run_bass_kernel_spmd(
        nc, in_maps, core_ids=list(range(8)), **(_run_kwargs or {})
    )
    bo = np.asarray(bo, np.float32)
    outp = np.empty((2, S, D), dtype=np.float32)
    for b in range(2):
        acc = res.results[4 * b]["out"].astype(np.float32)
        for g in range(1, 4):
            acc = acc + res.results[4 * b + g]["out"].astype(np.float32)
        outp[b] = acc + bo[None, :]
    kernel.last_result = res
    return outp


# revision 13
# speedup vs baseline: 1.0893x; 1.0718x over previous
"""Entropy-regularized attention (standard MHA fwd) on 8 trn2 cores.

Sharding: core c -> batch b=c//4, head-group g=c%4 (4 of 16 heads).
Each core computes q/k/v for its 256-wide head-group slice, transposed-
layout attention, then a row-split Wo partial product. Host sums the 4
partials per batch and adds bo (the "all-reduce").

v3 restructure vs v2 (261us -> target ~160us):
- Scores are emitted as ROW-TILED HEAD PAIRS: the two heads of an mt
  group have K=64 stationaries at base partitions 0 and 64, so their
  score matmuls auto-derive tile_position (0,0)/(64,0) and execute
  CONCURRENTLY on the PE (each K=64 tile uses half the array rows).
  This halves the PE time of the scores phase.
- QG=512 with 4 query groups: per pair-unit one [128,2,512] PSUM tile
  holds both heads' scores and ONE Exp activation (N=1024) covers the
  pair, keeping ACT's per-instruction overhead amortized while PSUM
  stays within 8 banks (sc 2x2 + av 2 + ops 2).
- Softmax normalization no longer uses ACT Ln/Exp nor PE broadcast
  matmuls: DVE reciprocal computes 1/l, GPSIMD partition_broadcast
  fans it across 64 partitions, DVE tensor_mul normalizes into avT.
  ACT does nothing but the exp stream; the PE does nothing but matmul.
- ALL non-score PE work (q/k/v projections, av accumulation, Wo
  partials) lives in deferred queues popped under a per-unit cost
  budget (~880ns) between score emissions, so the exp stream is never
  blocked by more than one unit's worth of filler. av matmuls are
  gated on their v-projection pieces to preserve FIFO progress.
- Unit order: pair 01 for all 4 qgs (kt-inner), then pair 23. kT/qT
  mt=1 projections amortize over the first 64 units; Wo partials for
  each qg unlock after its second pair's norms and fill the late
  units, leaving only qg3's Wo (+ final norm chain) in the tail.
"""

import sys

for _p in ("/opt/trn_rl_repo", "/root/.axon_site/_ro/trn_rl_repo"):
    if _p not in sys.path:
        sys.path.insert(0, _p)

import numpy as np

import concourse.bass as bass
import concourse.mybir as mybir
import concourse.tile as tile
from concourse import bacc

P = 128
S = 2048  # sequence length
D = 1024  # hidden
DG = 256  # per-core head-group width (4 heads x 64)
HD = 64
NHL = 4  # heads per core
KT_D = D // P  # 8 contraction tiles for projections
ST = S // P  # 16 sequence tiles
QG = 512  # query-group width
NQG = S // QG  # 4 query groups
NPR = 2  # head pairs per core

F32 = mybir.dt.float32
F32R = mybir.dt.float32r
BF16 = mybir.dt.bfloat16


def build_nc():
    nc = bacc.Bacc(None, target_bir_lowering=False)

    # all inputs pre-arranged on the host into the exact SBUF layouts so
    # every DMA is a contiguous-per-partition blob (large descriptors)
    xT = nc.dram_tensor("xT", [P, 4 * KT_D * 512], BF16, kind="ExternalInput")
    wq = nc.dram_tensor("wq", [P, KT_D * DG], BF16, kind="ExternalInput")
    wk = nc.dram_tensor("wk", [P, KT_D * DG], BF16, kind="ExternalInput")
    wv = nc.dram_tensor("wv", [P, KT_D * DG], BF16, kind="ExternalInput")
    wo = nc.dram_tensor("wo", [P, 2 * D], BF16, kind="ExternalInput")
    bq = nc.dram_tensor("bq", [P, 2], F32, kind="ExternalInput")
    bk = nc.dram_tensor("bk", [P, 2], F32, kind="ExternalInput")
    bv = nc.dram_tensor("bv", [1, DG], F32R, kind="ExternalInput")
    out = nc.dram_tensor("out", [S, D], BF16, kind="ExternalOutput")

    with tile.TileContext(nc) as tc:
        _body(tc, nc, xT, wq, wk, wv, wo, bq, bk, bv, out)

    # Pin Exp/Ln to the one table set holding both: strip them from the
    # competing sets (dict order and size preserved, so act_func_set_id
    # indices stay valid). Without this the table-load pass can bounce
    # between table sets.
    import concourse.bacc as _bacc_mod

    _orig_tables = _bacc_mod.get_activation_tables

    def _pinned_tables(arch):
        t = _orig_tables(arch)
        for name, fns in t.items():
            if name != "natural_log_exp_and_others":
                fns.discard(mybir.ActivationFunctionType.Exp)
                fns.discard(mybir.ActivationFunctionType.Ln)
        return t

    _bacc_mod.get_activation_tables = _pinned_tables
    try:
        nc.compile()
    finally:
        _bacc_mod.get_activation_tables = _orig_tables
    return nc


def _body(tc, nc, xT, wq, wk, wv, wo, bq, bk, bv, out):
    from collections import deque
    from contextlib import ExitStack

    with ExitStack() as ctx:
        ctx.enter_context(
            nc.allow_low_precision(
                reason="bf16 matmul inputs; accumulation is fp32 PSUM"
            )
        )
        persist = ctx.enter_context(tc.tile_pool(name="persist", bufs=1))
        expool = ctx.enter_context(tc.tile_pool(name="expool", bufs=16))
        ulpool = ctx.enter_context(tc.tile_pool(name="ulpool", bufs=12))
        rpool = ctx.enter_context(tc.tile_pool(name="rpool", bufs=4))
        l4pool = ctx.enter_context(tc.tile_pool(name="l4pool", bufs=4))
        r4pool = ctx.enter_context(tc.tile_pool(name="r4pool", bufs=2))
        opool = ctx.enter_context(tc.tile_pool(name="opool", bufs=3))
        # PSUM budget (8 banks): sc 2x[128,2,512]=4, av 2x[65,512]=2,
        # ops 2x[128,512]=2. qkv/v/proj/wo tiles all use the ops slots.
        ps_sc = ctx.enter_context(tc.tile_pool(name="ps_sc", bufs=2, space="PSUM"))
        ps_av = ctx.enter_context(tc.tile_pool(name="ps_av", bufs=2, space="PSUM"))
        ps_o = ctx.enter_context(tc.tile_pool(name="ps_o", bufs=2, space="PSUM"))

        qT_sb = persist.tile([P, 2, S], BF16)
        kT_sb = persist.tile([P, 2, S], BF16)
        v_sb = persist.tile([P, ST, NHL * 65], BF16)  # 65-striped: col 64 = ones
        avT = [
            persist.tile([P, 2, QG], BF16, tag=f"avT{g}", name=f"avT{g}")
            for g in range(NQG)
        ]
        wo_sb = persist.tile([P, 2, D], BF16)
        ones_row = persist.tile([1, P], F32R)
        xT_sb = persist.tile([P, 4, KT_D, 512], BF16)
        wq_sb = persist.tile([P, KT_D, DG], BF16, tag="wq")
        wk_sb = persist.tile([P, KT_D, DG], BF16, tag="wk")
        wv_sb = persist.tile([P, KT_D, DG], BF16, tag="wv")
        bq_sb = persist.tile([P, 2], F32, tag="bq")
        bk_sb = persist.tile([P, 2], F32, tag="bk")
        bv_sb = persist.tile([1, DG], F32R, tag="bv")

        # DMAs ordered to match first-use: wk then the first xT column
        # block gate the first projection chain.
        xTr = xT.rearrange("p (cb kt s) -> p cb kt s", kt=KT_D, s=512)
        nc.sync.dma_start(wk_sb[:], wk.rearrange("p (kt n) -> p kt n", n=DG))
        nc.sync.dma_start(xT_sb[:, 0, 0:4], xTr[:, 0, 0:4])
        nc.sync.dma_start(xT_sb[:, 0, 4:8], xTr[:, 0, 4:8])
        nc.sync.dma_start(bk_sb[:], bk[:])
        nc.sync.dma_start(bq_sb[:], bq[:])
        nc.sync.dma_start(wq_sb[:], wq.rearrange("p (kt n) -> p kt n", n=DG))
        nc.sync.dma_start(wv_sb[:], wv.rearrange("p (kt n) -> p kt n", n=DG))
        nc.sync.dma_start(bv_sb[:], bv[:])
        nc.sync.dma_start(xT_sb[:, 1], xTr[:, 1])
        nc.sync.dma_start(xT_sb[:, 2], xTr[:, 2])
        nc.sync.dma_start(xT_sb[:, 3], xTr[:, 3])
        nc.sync.dma_start(wo_sb[:], wo.rearrange("p (kt n) -> p kt n", n=D))

        # memset can't emit float32r; stage fp32 ones and copy-cast
        ones_f32 = persist.tile([P, P], F32)
        nc.vector.memset(ones_f32[:], 1.0)
        nc.vector.tensor_copy(ones_row[:], ones_f32[0:1, :])
        nc.vector.tensor_copy(
            v_sb.rearrange("p st (h w) -> p st h w", w=65)[:, :, :, 64],
            ones_f32[:, 0:64].rearrange("p (st h) -> p st h", h=4),
        )

        # ---- projection building blocks ----
        def qk_half(ps, wsb, mt, nq, kts):
            for kt in kts:
                nc.tensor.matmul(
                    ps[:, 0:512],
                    wsb[:, kt, mt * P : (mt + 1) * P],
                    xT_sb[:, nq, kt, :],
                    start=(kt == 0),
                    stop=(kt == KT_D - 1),
                )

        def qk_evict(ps, bsb, dest, mt, nq):
            nc.vector.tensor_scalar_add(
                dest[:, mt, nq * 512 : (nq + 1) * 512],
                ps[:, 0:512],
                bsb[:, mt : mt + 1],
            )

        # emission-order readiness flags: a score matmul may only be
        # EMITTED after the projection chain writing its qT/kT region has
        # been emitted (the Tile framework tracks writer->reader deps in
        # program order; a later-emitted writer would be a race).
        qk_ready = {}  # ("q"/"k", mt, nq) -> True

        def qk_full(wsb, bsb, dest, mt, nq, key):
            ps = ps_o.tile([P, 512], F32, tag="ops", name="qkps")
            qk_half(ps, wsb, mt, nq, range(0, KT_D))
            qk_evict(ps, bsb, dest, mt, nq)
            qk_ready[(key, mt, nq)] = True

        def qk_pieces(wsb, bsb, dest, mt, nq, key, pieces=4):
            # split the 8-matmul K-chain into `pieces` closures; the last
            # one carries the bias eviction. cost ~= (8/pieces)*213ns
            box = []
            step = KT_D // pieces

            def make(pi):
                def run():
                    if pi == 0:
                        box.append(
                            ps_o.tile([P, 512], F32, tag="ops", name="qkps")
                        )
                    ps = box[-1]
                    qk_half(ps, wsb, mt, nq, range(pi * step, (pi + 1) * step))
                    if pi == pieces - 1:
                        qk_evict(ps, bsb, dest, mt, nq)
                        box.pop()
                        qk_ready[(key, mt, nq)] = True

                return run

            return [(step * 215, make(pi)) for pi in range(pieces)]

        v_done = [False] * ST

        def v_group_half(st, half):
            # half 0: kts 0-3; half 1: kts 4-7 + bias + evict
            def run():
                if half == 0:
                    v_group_half.box[st] = ps_o.tile(
                        [P, 512], F32, tag="ops", name="vps"
                    )
                    ps = v_group_half.box[st]
                    for kt in range(0, 4):
                        nc.tensor.matmul(
                            ps[:, 0:DG],
                            xT_sb[:, st // 4, kt, (st % 4) * P : (st % 4 + 1) * P],
                            wv_sb[:, kt, :],
                            start=(kt == 0),
                            stop=False,
                        )
                else:
                    ps = v_group_half.box.pop(st)
                    for kt in range(4, KT_D):
                        nc.tensor.matmul(
                            ps[:, 0:DG],
                            xT_sb[:, st // 4, kt, (st % 4) * P : (st % 4 + 1) * P],
                            wv_sb[:, kt, :],
                            start=False,
                            stop=False,
                        )
                    nc.tensor.matmul(
                        ps[:, 0:DG],
                        ones_row[0:1, 0:P],
                        bv_sb[0:1, :],
                        start=False,
                        stop=True,
                    )
                    nc.vector.tensor_copy(
                        v_sb.rearrange("p st (h w) -> p st h w", w=65)[
                            :, st, :, 0:64
                        ],
                        ps[:, 0:DG].rearrange("p (h w) -> p h w", w=64),
                    )
                    v_done[st] = True

            return run

        v_group_half.box = {}

        def v_group_full(st):
            v_group_half(st, 0)()
            v_group_half(st, 1)()

        # ---- deferred machinery ----
        # avq: essential attention follow-up (av accumulation + norms),
        # popped first. defq: projection / Wo filler under a budget.
        avq = deque()
        defq = deque()
        pending_av = deque()  # av pieces waiting on their v group

        BUDGET = 880
        n_av_emitted = [0]  # count of av matmul pieces emitted (2 per unit)

        def drain_pending():
            while pending_av and v_done[pending_av[0][0]]:
                _, kind, cost, fn = pending_av.popleft()
                avq.append((kind, cost, fn))

        def pop_one_av():
            drain_pending()
            if avq:
                kind, _, fn = avq.popleft()
                fn()
                if kind == "av":
                    n_av_emitted[0] += 1
                return True
            return False

        def pop_one_def():
            if defq:
                _, fn = defq.popleft()
                fn()
                return True
            return False

        def pop_pieces():
            spent = 0
            drain_pending()
            while avq and spent < BUDGET:
                kind, cost, fn = avq.popleft()
                fn()
                if kind == "av":
                    n_av_emitted[0] += 1
                spent += cost
            while defq and spent < BUDGET:
                cost, fn = defq.popleft()
                fn()
                spent += cost

        def ensure_qk(key, mt, nq):
            # force-pop until the projection chain for this region has run
            while not qk_ready.get((key, mt, nq)):
                if not pop_one_def():
                    raise RuntimeError(f"deadlock: {key} mt{mt} nq{nq}")

        def ensure_av_through(unit_idx):
            # all av pieces of units <= unit_idx emitted (ex-slot reuse)
            while n_av_emitted[0] < 2 * (unit_idx + 1):
                if not pop_one_av() and not pop_one_def():
                    raise RuntimeError("deadlock: av drain")

        # ---- attention phase ----
        EXBUFS = 16  # must match expool bufs

        def sc_exp(pr, qg, kt):
            ensure_qk("k", pr, kt // 4)
            ensure_qk("q", pr, qg)
            mt, q0 = pr, qg * QG
            sc = ps_sc.tile([P, 2, 512], F32, tag="sc", name="sc")
            for hh in range(2):
                po = hh * 64
                nc.tensor.matmul(
                    sc[:, hh, :],
                    kT_sb[po : po + 64, mt, kt * P : (kt + 1) * P],
                    qT_sb[po : po + 64, mt, q0 : q0 + 512],
                    start=True,
                    stop=True,
                )
            ex = expool.tile([P, 2, 512], BF16, tag="ex", name="ex")
            nc.scalar.activation(
                ex[:], sc[:], mybir.ActivationFunctionType.Exp, scale=0.125
            )
            return ex

        avps = {}  # hh -> live av psum tile for current (pr, qg)

        def av_piece(pr, qg, kt, hh, ex):
            h = 2 * pr + hh

            def run():
                if kt == 0:
                    avps[hh] = ps_av.tile([65, 512], F32, tag="av", name="av")
                nc.tensor.matmul(
                    avps[hh][0:65, :],
                    v_sb[:, kt, h * 65 : h * 65 + 65],
                    ex[:, hh, :],
                    start=(kt == 0),
                    stop=(kt == ST - 1),
                )

            return (215, run)

        # Softmax normalization: per (pr,qg,head) the av PSUM is copied to
        # SBUF (ul) and its denominator row gathered into l4[qg] at
        # partition 32h. After the SECOND pair of a qg lands, ONE batched
        # Ln + ONE Exp(-x) on ACT produce 1/l for all 4 heads ([97,512]
        # costs the same as [1,512]); GPSIMD broadcasts each head's row
        # across 64 partitions and DVE multiplies into avT. avT is only
        # read by Wo which needs both pairs anyway, so deferring pair-0's
        # normalize to the qg norm costs nothing.
        uls = {}
        l4s = {}

        def norm_pieces(pr, qg, hh, last):
            mt, po = pr, hh * 64
            h = 2 * pr + hh

            def p_ul():
                av = avps.pop(hh)
                if (qg not in l4s) and pr == 0 and hh == 0:
                    l4s[qg] = l4pool.tile([97, 512], F32, tag="l4", name="l4")
                    nc.gpsimd.memset(l4s[qg][:], 1.0)
                ul = ulpool.tile([65, 512], F32, tag="ul", name="ul")
                nc.vector.tensor_copy(ul[:], av[0:65, :])
                nc.vector.tensor_copy(
                    l4s[qg][32 * h : 32 * h + 1, :], av[64:65, :]
                )
                uls[(qg, h)] = ul

            pieces = [(50, p_ul)]
            if last:

                def p_norm():
                    # 1/l = exp(-ln(l)) for all 4 heads in one Ln + one Exp
                    l4 = l4s.pop(qg)
                    ls = r4pool.tile([97, 512], F32, tag="ls", name="ls")
                    nc.scalar.activation(
                        ls[:], l4[:], mybir.ActivationFunctionType.Ln
                    )
                    r4 = r4pool.tile([97, 512], F32, tag="r4", name="r4")
                    nc.scalar.activation(
                        r4[:], ls[:], mybir.ActivationFunctionType.Exp,
                        scale=-1.0,
                    )
                    norm_pieces.r4 = r4

                pieces.append((50, p_norm))

                def make_mul(h2):
                    def p_mul():
                        mt2, po2 = h2 // 2, (h2 % 2) * 64
                        ul = uls.pop((qg, h2))
                        # partition_broadcast reads physical partition 0:
                        # stage this head's reciprocal row to a base-0 tile
                        r1 = rpool.tile([1, 512], F32, tag="r1", name="r1")
                        nc.vector.tensor_copy(
                            r1[:], norm_pieces.r4[32 * h2 : 32 * h2 + 1, :]
                        )
                        rbb = rpool.tile([64, 512], F32, tag="rbb", name="rbb")
                        nc.gpsimd.partition_broadcast(
                            rbb[:], r1[:], channels=64
                        )
                        nc.vector.tensor_mul(
                            out=avT[qg][po2 : po2 + 64, mt2, :],
                            in0=ul[0:64, :],
                            in1=rbb[:],
                        )
                        if h2 == 3:
                            defq.extend(wo_pieces(qg))

                    return p_mul

                for h2 in range(4):
                    pieces.append((50, make_mul(h2)))
            return pieces

        ot_box = {}

        def wo_pieces(qg):
            # per (sti, nd): 2 matmuls (kt2 accumulation) + eviction; the
            # ops-pool slot is alloc'd and freed within one piece so the
            # bufs=2 rotation can interleave with qk/v pieces.
            pieces = []

            def make(sti, nd):
                def run():
                    st = qg * (QG // P) + sti
                    if nd == 0:
                        ot_box[sti] = opool.tile([P, D], BF16, tag="ot", name="ot")
                    ot = ot_box[sti]
                    pp = ps_o.tile([P, 512], F32, tag="ops", name="pp")
                    for kt2 in range(2):
                        nc.tensor.matmul(
                            pp[:],
                            avT[qg][:, kt2, sti * P : (sti + 1) * P],
                            wo_sb[:, kt2, nd * 512 : (nd + 1) * 512],
                            start=(kt2 == 0),
                            stop=(kt2 == 1),
                        )
                    nc.vector.tensor_copy(ot[:, nd * 512 : (nd + 1) * 512], pp[:])
                    if nd == 1:
                        del ot_box[sti]
                        nc.sync.dma_start(out[st * P : (st + 1) * P, :], ot[:])

                return run

            for sti in range(QG // P):
                for nd in range(2):
                    pieces.append((460, make(sti, nd)))
            return pieces

        # ---- upfront phase (continuous PE stretch; overlaps input DMA) ----
        qk_full(wk_sb, bk_sb, kT_sb, 0, 0, "k")
        qk_full(wq_sb, bq_sb, qT_sb, 0, 0, "q")
        v_group_full(0)
        v_group_full(1)

        # ---- static filler: ordered by need-by unit ----
        # kT mt0 fully by unit ~12 (sc consumes kt blocks 4/8/12 at units
        # 4/8/12); q(mt0,nq1..3) by units 16/32/48; v(st) before av(st)
        # emission (forced by ensure_av_through); mt1 chains by unit 64.
        defq.extend(qk_pieces(wk_sb, bk_sb, kT_sb, 0, 1, "k"))
        defq.extend(qk_pieces(wk_sb, bk_sb, kT_sb, 0, 2, "k"))
        defq.extend(qk_pieces(wk_sb, bk_sb, kT_sb, 0, 3, "k"))
        defq.extend(qk_pieces(wq_sb, bq_sb, qT_sb, 0, 1, "q"))
        for st in (2, 3, 4, 5):
            defq.append((500, v_group_half(st, 0)))
            defq.append((560, v_group_half(st, 1)))
        defq.extend(qk_pieces(wq_sb, bq_sb, qT_sb, 0, 2, "q"))
        for st in (6, 7, 8, 9):
            defq.append((500, v_group_half(st, 0)))
            defq.append((560, v_group_half(st, 1)))
        defq.extend(qk_pieces(wq_sb, bq_sb, qT_sb, 0, 3, "q"))
        for st in (10, 11, 12, 13, 14, 15):
            defq.append((500, v_group_half(st, 0)))
            defq.append((560, v_group_half(st, 1)))
        for nq in range(4):
            defq.extend(qk_pieces(wk_sb, bk_sb, kT_sb, 1, nq, "k"))
            defq.extend(qk_pieces(wq_sb, bq_sb, qT_sb, 1, nq, "q"))

        # ---- the unit loop ----
        units = [
            (pr, qg, kt)
            for pr in range(NPR)
            for qg in range(NQG)
            for kt in range(ST)
        ]
        ex_tiles = {0: sc_exp(*units[0])}
        for i, (pr, qg, kt) in enumerate(units):
            if i + 1 < len(units):
                ensure_av_through(i + 1 - EXBUFS)
                ex_tiles[i + 1] = sc_exp(*units[i + 1])
            ex = ex_tiles.pop(i)
            for hh in range(2):
                pending_av.append((kt, "av", *av_piece(pr, qg, kt, hh, ex)))
            if kt == ST - 1:
                for hh in range(2):
                    for piece in norm_pieces(pr, qg, hh, pr == NPR - 1 and hh == 1):
                        pending_av.append((kt, "norm", *piece))
            pop_pieces()
        # ---- tail: drain everything ----
        while pending_av or avq or defq:
            if not pop_one_av():
                if not pop_one_def():
                    raise RuntimeError("tail deadlock")


_NC_CACHE = None


def get_nc():
    global _NC_CACHE
    if _NC_CACHE is None:
        _NC_CACHE = build_nc()
    return _NC_CACHE


def make_in_maps(x, Wq, bq, Wk, bk, Wv, bv, Wo, bo):
    import ml_dtypes

    bf16 = ml_dtypes.bfloat16

    def w_arr(W, sl):
        # [D, DG] -> [p, kt*DG]: W[kt*128+p, n] at [p, kt, n]
        return np.ascontiguousarray(
            W[:, sl].reshape(KT_D, P, DG).transpose(1, 0, 2).reshape(P, -1)
        ).astype(bf16)

    in_maps = []
    for c in range(8):
        b, g = c // 4, c % 4
        sl = slice(g * DG, (g + 1) * DG)
        # x[b].T is [D, S]; SBUF wants [p, cb, kt, 512] with row kt*128+p,
        # col cb*512+s
        xt = (
            x[b]
            .T.reshape(KT_D, P, 4, 512)
            .transpose(1, 2, 0, 3)
            .reshape(P, -1)
            .astype(bf16)
        )
        wo_a = np.ascontiguousarray(
            Wo[sl, :].reshape(2, P, D).transpose(1, 0, 2).reshape(P, -1)
        ).astype(bf16)
        in_maps.append(
            {
                "xT": np.ascontiguousarray(xt),
                "wq": w_arr(Wq, sl),
                "wk": w_arr(Wk, sl),
                "wv": w_arr(Wv, sl),
                "wo": wo_a,
                "bq": np.ascontiguousarray(bq[sl].reshape(2, P).T),
                "bk": np.ascontiguousarray(bk[sl].reshape(2, P).T),
                "bv": np.ascontiguousarray(bv[sl].reshape(1, DG)),
            }
        )
    return in_maps


def kernel(x, Wq, bq, Wk, bk, Wv, bv, Wo, bo, _run_kwargs=None):
    from concourse.bass_utils import run_bass_kernel_spmd

    x = np.asarray(x, dtype=np.float32)
    nc = get_nc()
    in_maps = make_in_maps(
        x,
        np.asarray(Wq, np.float32),
        np.asarray(bq, np.float32),
        np.asarray(Wk, np.float32),
        np.asarray(bk, np.float32),
        np.asarray(Wv, np.float32),
        np.asarray(bv, np.float32),
        np.asarray(Wo, np.float32),
        np.asarray(bo, np.float32),
    )
    res = run_bass_kernel_spmd(
        nc, in_maps, core_ids=list(range(8)), **(_run_kwargs or {})
    )
    bo = np.asarray(bo, np.float32)
    outp = np.empty((2, S, D), dtype=np.float32)
    for b in range(2):
        acc = res.results[4 * b]["out"].astype(np.float32)
        for g in range(1, 4):
            acc = acc + res.results[4 * b + g]["out"].astype(np.float32)
        outp[b] = acc + bo[None, :]
    kernel.last_result = res
    return outp


# revision 18
# speedup vs baseline: 1.1229x; 1.0308x over previous
"""Entropy-regularized attention (standard MHA fwd) on 8 trn2 cores.

Sharding: core c -> batch b=c//4, head-group g=c%4 (4 of 16 heads).
Each core computes q/k/v for its 256-wide head-group slice, transposed-
layout attention, then a row-split Wo partial product. Host sums the 4
partials per batch and adds bo (the "all-reduce").

v3 restructure vs v2 (261us -> target ~160us):
- Scores are emitted as ROW-TILED HEAD PAIRS: the two heads of an mt
  group have K=64 stationaries at base partitions 0 and 64, so their
  score matmuls auto-derive tile_position (0,0)/(64,0) and execute
  CONCURRENTLY on the PE (each K=64 tile uses half the array rows).
  This halves the PE time of the scores phase.
- QG=512 with 4 query groups: per pair-unit one [128,2,512] PSUM tile
  holds both heads' scores and ONE Exp activation (N=1024) covers the
  pair, keeping ACT's per-instruction overhead amortized while PSUM
  stays within 8 banks (sc 2x2 + av 2 + ops 2).
- Softmax normalization no longer uses ACT Ln/Exp nor PE broadcast
  matmuls: DVE reciprocal computes 1/l, GPSIMD partition_broadcast
  fans it across 64 partitions, DVE tensor_mul normalizes into avT.
  ACT does nothing but the exp stream; the PE does nothing but matmul.
- ALL non-score PE work (q/k/v projections, av accumulation, Wo
  partials) lives in deferred queues popped under a per-unit cost
  budget (~880ns) between score emissions, so the exp stream is never
  blocked by more than one unit's worth of filler. av matmuls are
  gated on their v-projection pieces to preserve FIFO progress.
- Unit order: pair 01 for all 4 qgs (kt-inner), then pair 23. kT/qT
  mt=1 projections amortize over the first 64 units; Wo partials for
  each qg unlock after its second pair's norms and fill the late
  units, leaving only qg3's Wo (+ final norm chain) in the tail.
"""

import sys

for _p in ("/opt/trn_rl_repo", "/root/.axon_site/_ro/trn_rl_repo"):
    if _p not in sys.path:
        sys.path.insert(0, _p)

import numpy as np

import concourse.bass as bass
import concourse.mybir as mybir
import concourse.tile as tile
from concourse import bacc

P = 128
S = 2048  # sequence length
D = 1024  # hidden
DG = 256  # per-core head-group width (4 heads x 64)
HD = 64
NHL = 4  # heads per core
KT_D = D // P  # 8 contraction tiles for projections
ST = S // P  # 16 sequence tiles
QG = 512  # query-group width
NQG = S // QG  # 4 query groups
NPR = 2  # head pairs per core

F32 = mybir.dt.float32
F32R = mybir.dt.float32r
BF16 = mybir.dt.bfloat16


def build_nc():
    nc = bacc.Bacc(None, target_bir_lowering=False)

    # all inputs pre-arranged on the host into the exact SBUF layouts so
    # every DMA is a contiguous-per-partition blob (large descriptors)
    xT = nc.dram_tensor("xT", [P, 4 * KT_D * 512], BF16, kind="ExternalInput")
    wq = nc.dram_tensor("wq", [P, KT_D * DG], BF16, kind="ExternalInput")
    wk = nc.dram_tensor("wk", [P, KT_D * DG], BF16, kind="ExternalInput")
    wv = nc.dram_tensor("wv", [P, KT_D * DG], BF16, kind="ExternalInput")
    wo = nc.dram_tensor("wo", [P, 2 * D], BF16, kind="ExternalInput")
    bq = nc.dram_tensor("bq", [P, 2], F32, kind="ExternalInput")
    bk = nc.dram_tensor("bk", [P, 2], F32, kind="ExternalInput")
    bv = nc.dram_tensor("bv", [1, DG], F32R, kind="ExternalInput")
    out = nc.dram_tensor("out", [S, D], BF16, kind="ExternalOutput")

    with tile.TileContext(nc) as tc:
        _body(tc, nc, xT, wq, wk, wv, wo, bq, bk, bv, out)

    # Pin Exp/Ln to the one table set holding both: strip them from the
    # competing sets (dict order and size preserved, so act_func_set_id
    # indices stay valid). Without this the table-load pass can bounce
    # between table sets.
    import concourse.bacc as _bacc_mod

    _orig_tables = _bacc_mod.get_activation_tables

    def _pinned_tables(arch):
        t = _orig_tables(arch)
        for name, fns in t.items():
            if name != "natural_log_exp_and_others":
                fns.discard(mybir.ActivationFunctionType.Exp)
                fns.discard(mybir.ActivationFunctionType.Ln)
        return t

    _bacc_mod.get_activation_tables = _pinned_tables
    try:
        nc.compile()
    finally:
        _bacc_mod.get_activation_tables = _orig_tables
    return nc


def _body(tc, nc, xT, wq, wk, wv, wo, bq, bk, bv, out):
    from collections import deque
    from contextlib import ExitStack

    with ExitStack() as ctx:
        ctx.enter_context(
            nc.allow_low_precision(
                reason="bf16 matmul inputs; accumulation is fp32 PSUM"
            )
        )
        persist = ctx.enter_context(tc.tile_pool(name="persist", bufs=1))
        expool = ctx.enter_context(tc.tile_pool(name="expool", bufs=16))
        ulpool = ctx.enter_context(tc.tile_pool(name="ulpool", bufs=12))
        rpool = ctx.enter_context(tc.tile_pool(name="rpool", bufs=4))
        l4pool = ctx.enter_context(tc.tile_pool(name="l4pool", bufs=4))
        r4pool = ctx.enter_context(tc.tile_pool(name="r4pool", bufs=2))
        opool = ctx.enter_context(tc.tile_pool(name="opool", bufs=3))
        # PSUM budget (8 banks): sc 2x[128,2,512]=4, av 2x[65,512]=2,
        # ops 2x[128,512]=2. qkv/v/proj/wo tiles all use the ops slots.
        ps_sc = ctx.enter_context(tc.tile_pool(name="ps_sc", bufs=2, space="PSUM"))
        ps_av = ctx.enter_context(tc.tile_pool(name="ps_av", bufs=2, space="PSUM"))
        ps_o = ctx.enter_context(tc.tile_pool(name="ps_o", bufs=2, space="PSUM"))

        qT_sb = persist.tile([P, 2, S], BF16)
        kT_sb = persist.tile([P, 2, S], BF16)
        v_sb = persist.tile([P, ST, NHL * 65], BF16)  # 65-striped: col 64 = ones
        avT = [
            persist.tile([P, 2, QG], BF16, tag=f"avT{g}", name=f"avT{g}")
            for g in range(NQG)
        ]
        wo_sb = persist.tile([P, 2, D], BF16)
        ones_row = persist.tile([1, P], F32R)
        xT_sb = persist.tile([P, 4, KT_D, 512], BF16)
        wq_sb = persist.tile([P, KT_D, DG], BF16, tag="wq")
        wk_sb = persist.tile([P, KT_D, DG], BF16, tag="wk")
        wv_sb = persist.tile([P, KT_D, DG], BF16, tag="wv")
        bq_sb = persist.tile([P, 2], F32, tag="bq")
        bk_sb = persist.tile([P, 2], F32, tag="bk")
        bv_sb = persist.tile([1, DG], F32R, tag="bv")

        # DMAs ordered to match first-use: wk then the first xT column
        # block gate the first projection chain.
        xTr = xT.rearrange("p (cb kt s) -> p cb kt s", kt=KT_D, s=512)
        nc.sync.dma_start(wk_sb[:], wk.rearrange("p (kt n) -> p kt n", n=DG))
        nc.sync.dma_start(xT_sb[:, 0, 0:4], xTr[:, 0, 0:4])
        nc.sync.dma_start(bk_sb[:], bk[:])
        nc.sync.dma_start(wq_sb[:], wq.rearrange("p (kt n) -> p kt n", n=DG))
        nc.sync.dma_start(xT_sb[:, 0, 4:8], xTr[:, 0, 4:8])
        nc.sync.dma_start(bq_sb[:], bq[:])
        nc.sync.dma_start(wv_sb[:], wv.rearrange("p (kt n) -> p kt n", n=DG))
        nc.sync.dma_start(bv_sb[:], bv[:])
        nc.sync.dma_start(xT_sb[:, 1], xTr[:, 1])
        nc.sync.dma_start(xT_sb[:, 2], xTr[:, 2])
        nc.sync.dma_start(xT_sb[:, 3], xTr[:, 3])
        nc.sync.dma_start(wo_sb[:], wo.rearrange("p (kt n) -> p kt n", n=D))

        # memset can't emit float32r; stage fp32 ones and copy-cast
        ones_f32 = persist.tile([P, P], F32)
        nc.vector.memset(ones_f32[:], 1.0)
        nc.vector.tensor_copy(ones_row[:], ones_f32[0:1, :])
        ones_all = persist.tile([P, P], BF16)
        nc.vector.tensor_copy(ones_all[:], ones_f32[:])
        nc.vector.tensor_copy(
            v_sb.rearrange("p st (h w) -> p st h w", w=65)[:, :, :, 64],
            ones_f32[:, 0:64].rearrange("p (st h) -> p st h", h=4),
        )

        # ---- projection building blocks ----
        def qk_half(ps, wsb, mt, nq, kts):
            for kt in kts:
                nc.tensor.matmul(
                    ps[:, 0:512],
                    wsb[:, kt, mt * P : (mt + 1) * P],
                    xT_sb[:, nq, kt, :],
                    start=(kt == 0),
                    stop=(kt == KT_D - 1),
                )

        def qk_evict(ps, bsb, dest, mt, nq):
            nc.vector.tensor_scalar_add(
                dest[:, mt, nq * 512 : (nq + 1) * 512],
                ps[:, 0:512],
                bsb[:, mt : mt + 1],
            )

        # emission-order readiness flags: a score matmul may only be
        # EMITTED after the projection chain writing its qT/kT region has
        # been emitted (the Tile framework tracks writer->reader deps in
        # program order; a later-emitted writer would be a race).
        qk_ready = {}  # ("q"/"k", mt, nq) -> True

        def qk_full(wsb, bsb, dest, mt, nq, key):
            ps = ps_o.tile([P, 512], F32, tag="ops", name="qkps")
            qk_half(ps, wsb, mt, nq, range(0, KT_D))
            qk_evict(ps, bsb, dest, mt, nq)
            qk_ready[(key, mt, nq)] = True

        def qk_pieces(wsb, bsb, dest, mt, nq, key, pieces=4):
            # split the 8-matmul K-chain into `pieces` closures; the last
            # one carries the bias eviction. cost ~= (8/pieces)*213ns
            box = []
            step = KT_D // pieces

            def make(pi):
                def run():
                    if pi == 0:
                        box.append(
                            ps_o.tile([P, 512], F32, tag="ops", name="qkps")
                        )
                    ps = box[-1]
                    qk_half(ps, wsb, mt, nq, range(pi * step, (pi + 1) * step))
                    if pi == pieces - 1:
                        qk_evict(ps, bsb, dest, mt, nq)
                        box.pop()
                        qk_ready[(key, mt, nq)] = True

                return run

            return [(step * 215, make(pi)) for pi in range(pieces)]

        v_done = [False] * ST

        def v_group_half(st, half):
            # half 0: kts 0-3; half 1: kts 4-7 + bias + evict
            def run():
                if half == 0:
                    v_group_half.box[st] = ps_o.tile(
                        [P, 512], F32, tag="ops", name="vps"
                    )
                    ps = v_group_half.box[st]
                    for kt in range(0, 4):
                        nc.tensor.matmul(
                            ps[:, 0:DG],
                            xT_sb[:, st // 4, kt, (st % 4) * P : (st % 4 + 1) * P],
                            wv_sb[:, kt, :],
                            start=(kt == 0),
                            stop=False,
                        )
                else:
                    ps = v_group_half.box.pop(st)
                    for kt in range(4, KT_D):
                        nc.tensor.matmul(
                            ps[:, 0:DG],
                            xT_sb[:, st // 4, kt, (st % 4) * P : (st % 4 + 1) * P],
                            wv_sb[:, kt, :],
                            start=False,
                            stop=False,
                        )
                    nc.tensor.matmul(
                        ps[:, 0:DG],
                        ones_row[0:1, 0:P],
                        bv_sb[0:1, :],
                        start=False,
                        stop=True,
                    )
                    nc.vector.tensor_copy(
                        v_sb.rearrange("p st (h w) -> p st h w", w=65)[
                            :, st, :, 0:64
                        ],
                        ps[:, 0:DG].rearrange("p (h w) -> p h w", w=64),
                    )
                    v_done[st] = True

            return run

        v_group_half.box = {}

        def v_group_full(st):
            v_group_half(st, 0)()
            v_group_half(st, 1)()

        # ---- deferred machinery ----
        # avq: essential attention follow-up (av accumulation + norms),
        # popped first. defq: projection / Wo filler under a budget.
        avq = deque()
        defq = deque()
        pending_av = deque()  # av pieces waiting on their v group

        BUDGET = 880
        n_av_emitted = [0]  # count of av matmul pieces emitted (2 per unit)

        def drain_pending():
            while pending_av and v_done[pending_av[0][0]]:
                _, kind, cost, fn = pending_av.popleft()
                avq.append((kind, cost, fn))

        def pop_one_av():
            drain_pending()
            if avq:
                kind, _, fn = avq.popleft()
                fn()
                if kind == "av":
                    n_av_emitted[0] += 1
                return True
            return False

        def pop_one_def():
            if defq:
                _, fn = defq.popleft()
                fn()
                return True
            return False

        def pop_pieces():
            spent = 0
            drain_pending()
            while avq and spent < BUDGET:
                kind, cost, fn = avq.popleft()
                fn()
                if kind == "av":
                    n_av_emitted[0] += 1
                spent += cost
            while defq and spent < BUDGET:
                cost, fn = defq.popleft()
                fn()
                spent += cost

        def ensure_qk(key, mt, nq):
            # force-pop until the projection chain for this region has run
            while not qk_ready.get((key, mt, nq)):
                if not pop_one_def():
                    raise RuntimeError(f"deadlock: {key} mt{mt} nq{nq}")

        def ensure_av_through(unit_idx):
            # all av pieces of units <= unit_idx emitted (ex-slot reuse)
            while n_av_emitted[0] < 2 * (unit_idx + 1):
                if not pop_one_av() and not pop_one_def():
                    raise RuntimeError("deadlock: av drain")

        # ---- attention phase ----
        EXBUFS = 16  # must match expool bufs

        def sc_exp(pr, qg, kt):
            ensure_qk("k", pr, kt // 4)
            ensure_qk("q", pr, qg)
            mt, q0 = pr, qg * QG
            sc = ps_sc.tile([P, 2, 512], F32, tag="sc", name="sc")
            for hh in range(2):
                po = hh * 64
                nc.tensor.matmul(
                    sc[:, hh, :],
                    kT_sb[po : po + 64, mt, kt * P : (kt + 1) * P],
                    qT_sb[po : po + 64, mt, q0 : q0 + 512],
                    start=True,
                    stop=True,
                )
            ex = expool.tile([P, 2, 512], BF16, tag="ex", name="ex")
            nc.scalar.activation(
                ex[:], sc[:], mybir.ActivationFunctionType.Exp, scale=0.125
            )
            return ex

        avps = {}  # hh -> live av psum tile for current (pr, qg)

        def av_piece(pr, qg, kt, hh, ex):
            h = 2 * pr + hh

            def run():
                if kt == 0:
                    avps[hh] = ps_av.tile([65, 512], F32, tag="av", name="av")
                nc.tensor.matmul(
                    avps[hh][0:65, :],
                    v_sb[:, kt, h * 65 : h * 65 + 65],
                    ex[:, hh, :],
                    start=(kt == 0),
                    stop=(kt == ST - 1),
                )

            return (215, run)

        # Softmax normalization: per (pr,qg,head) the av PSUM is copied to
        # SBUF (ul) and its denominator row gathered into l4[qg] at
        # partition 32h. Once a qg's heads are all gathered, ONE batched
        # Ln + ONE Exp(-x) on ACT produce 1/l ([97,512] costs the same as
        # [1,512]); GPSIMD broadcasts each head's row across 64 partitions
        # and DVE multiplies into avT. The Ln/Exp+muls are DELAYED ~4
        # units so they land in ACT's FIFO behind already-ready exps
        # (emitting them immediately stalls the exp stream on the av->ul->
        # gather dependency chain). The LAST qg's second pair instead runs
        # per-head Ln/Exp straight from PSUM with a PE-matmul broadcast,
        # minimizing the serial tail after the final exp.
        uls = {}
        l4s = {}
        r4box = {}

        def p_ul_make(pr, qg, hh, skip_l=False):
            h = 2 * pr + hh

            def p_ul():
                av = avps.pop(hh)
                if qg not in l4s:
                    l4s[qg] = l4pool.tile([97, 512], F32, tag="l4", name="l4")
                    nc.gpsimd.memset(l4s[qg][:], 1.0)
                ul = ulpool.tile([65, 512], F32, tag="ul", name="ul")
                nc.vector.tensor_copy(ul[:], av[0:65, :])
                if not skip_l:
                    nc.vector.tensor_copy(
                        l4s[qg][32 * h : 32 * h + 1, :], av[64:65, :]
                    )
                uls[(qg, h)] = ul

            return p_ul

        def p_norm_make(qg, hi):
            def p_norm():
                # 1/l = exp(-ln(l)) for heads 0..hi/32 in one Ln + one Exp
                l4 = l4s.pop(qg)
                ls = r4pool.tile([97, 512], F32, tag="ls", name="ls")
                nc.scalar.activation(
                    ls[0:hi], l4[0:hi], mybir.ActivationFunctionType.Ln
                )
                r4 = r4pool.tile([97, 512], F32, tag="r4", name="r4")
                nc.scalar.activation(
                    r4[0:hi], ls[0:hi],
                    mybir.ActivationFunctionType.Exp, scale=-1.0,
                )
                r4box[qg] = r4

            return p_norm

        def p_mul_make(qg, h2, unlock_wo=False):
            def p_mul():
                mt2, po2 = h2 // 2, (h2 % 2) * 64
                ul = uls.pop((qg, h2))
                # partition_broadcast reads physical partition 0: stage
                # this head's reciprocal row to a base-0 tile first
                r1 = rpool.tile([1, 512], F32, tag="r1", name="r1")
                nc.vector.tensor_copy(
                    r1[:], r4box[qg][32 * h2 : 32 * h2 + 1, :]
                )
                rbb = rpool.tile([64, 512], F32, tag="rbb", name="rbb")
                nc.gpsimd.partition_broadcast(rbb[:], r1[:], channels=64)
                nc.vector.tensor_mul(
                    out=avT[qg][po2 : po2 + 64, mt2, :],
                    in0=ul[0:64, :],
                    in1=rbb[:],
                )
                if unlock_wo:
                    defq.extend(wo_pieces(qg))

            return p_mul

        def tail_head_pieces(qg, hh):
            # last qg, second pair: Ln/Exp straight from the av PSUM, PE
            # rb-matmul broadcast (PE is idle in the tail), DVE multiply.
            h = 2 + hh
            box = {}

            def p_lnexp():
                av = avps[hh]  # keep psum live; popped in p_ul below
                lnr = rpool.tile([1, 512], F32, tag="lnr", name="lnr")
                nc.scalar.activation(
                    lnr[:], av[64:65, :], mybir.ActivationFunctionType.Ln
                )
                r1 = rpool.tile([1, 512], BF16, tag="r1b", name="r1b")
                nc.scalar.activation(
                    r1[:], lnr[:], mybir.ActivationFunctionType.Exp,
                    scale=-1.0,
                )
                box["r1"] = r1

            def p_ul():
                av = avps.pop(hh)
                ul = ulpool.tile([65, 512], F32, tag="ul", name="ul")
                nc.vector.tensor_copy(ul[0:64, :], av[0:64, :])
                box["ul"] = ul

            def p_rbmul():
                rb = ps_o.tile([P, 512], F32, tag="ops", name="rb")
                nc.tensor.matmul(
                    rb[0:64, :],
                    ones_all[0:1, 0:64],
                    box["r1"][:],
                    start=True,
                    stop=True,
                )
                nc.vector.tensor_mul(
                    out=avT[qg][hh * 64 : hh * 64 + 64, 1, :],
                    in0=box["ul"][0:64, :],
                    in1=rb[0:64, :],
                )
                if hh == 1:
                    defq.extend(wo_pieces(qg))

            return [(50, p_lnexp), (50, p_ul), (250, p_rbmul)]

        ot_box = {}

        def wo_pieces(qg):
            # per (sti, nd): 2 matmuls (kt2 accumulation) + eviction; the
            # ops-pool slot is alloc'd and freed within one piece so the
            # bufs=2 rotation can interleave with qk/v pieces.
            pieces = []

            def make(sti, nd):
                def run():
                    st = qg * (QG // P) + sti
                    if nd == 0:
                        ot_box[sti] = opool.tile([P, D], BF16, tag="ot", name="ot")
                    ot = ot_box[sti]
                    pp = ps_o.tile([P, 512], F32, tag="ops", name="pp")
                    for kt2 in range(2):
                        nc.tensor.matmul(
                            pp[:],
                            avT[qg][:, kt2, sti * P : (sti + 1) * P],
                            wo_sb[:, kt2, nd * 512 : (nd + 1) * 512],
                            start=(kt2 == 0),
                            stop=(kt2 == 1),
                        )
                    nc.vector.tensor_copy(ot[:, nd * 512 : (nd + 1) * 512], pp[:])
                    if nd == 1:
                        del ot_box[sti]
                        nc.sync.dma_start(out[st * P : (st + 1) * P, :], ot[:])

                return run

            for sti in range(QG // P):
                for nd in range(2):
                    pieces.append((460, make(sti, nd)))
            return pieces

        # ---- upfront phase: only what the FIRST sc/exp needs (k+q mt0
        # nq0); v groups are deferred (av lags behind the exp stream) ----
        qk_full(wk_sb, bk_sb, kT_sb, 0, 0, "k")
        qk_full(wq_sb, bq_sb, qT_sb, 0, 0, "q")

        # ---- static filler: ordered by need-by unit ----
        # kT mt0 fully by unit ~12 (sc consumes kt blocks 4/8/12 at units
        # 4/8/12); q(mt0,nq1..3) by units 16/32/48; v(st) before av(st)
        # emission (forced by ensure_av_through); mt1 chains by unit 64.
        for st in (0, 1):
            defq.append((500, v_group_half(st, 0)))
            defq.append((560, v_group_half(st, 1)))
        defq.extend(qk_pieces(wk_sb, bk_sb, kT_sb, 0, 1, "k"))
        defq.extend(qk_pieces(wk_sb, bk_sb, kT_sb, 0, 2, "k"))
        defq.extend(qk_pieces(wk_sb, bk_sb, kT_sb, 0, 3, "k"))
        defq.extend(qk_pieces(wq_sb, bq_sb, qT_sb, 0, 1, "q"))
        for st in (2, 3, 4, 5):
            defq.append((500, v_group_half(st, 0)))
            defq.append((560, v_group_half(st, 1)))
        defq.extend(qk_pieces(wq_sb, bq_sb, qT_sb, 0, 2, "q"))
        for st in (6, 7, 8, 9):
            defq.append((500, v_group_half(st, 0)))
            defq.append((560, v_group_half(st, 1)))
        defq.extend(qk_pieces(wq_sb, bq_sb, qT_sb, 0, 3, "q"))
        for st in (10, 11, 12, 13, 14, 15):
            defq.append((500, v_group_half(st, 0)))
            defq.append((560, v_group_half(st, 1)))
        for nq in range(4):
            defq.extend(qk_pieces(wk_sb, bk_sb, kT_sb, 1, nq, "k"))
            defq.extend(qk_pieces(wq_sb, bq_sb, qT_sb, 1, nq, "q"))

        # ---- the unit loop ----
        units = [
            (pr, qg, kt)
            for pr in range(NPR)
            for qg in range(NQG)
            for kt in range(ST)
        ]
        delayed = []  # (release_iter, kind, cost, fn)

        ex_tiles = {0: sc_exp(*units[0])}
        for i, (pr, qg, kt) in enumerate(units):
            still = [e for e in delayed if e[0] > i]
            for _, kind, cost, fn in delayed:
                if _ <= i:
                    avq.append((kind, cost, fn))
            delayed[:] = still
            if i + 1 < len(units):
                ensure_av_through(i + 1 - EXBUFS)
                ex_tiles[i + 1] = sc_exp(*units[i + 1])
            ex = ex_tiles.pop(i)
            for hh in range(2):
                pending_av.append((kt, "av", *av_piece(pr, qg, kt, hh, ex)))
            if kt == ST - 1:
                last_qg = qg == NQG - 1
                if pr == 0:
                    for hh in range(2):
                        pending_av.append(
                            (kt, "norm", 50, p_ul_make(pr, qg, hh))
                        )
                    if last_qg:
                        # qg3 pair-0 heads: batched 2-head norm, delayed
                        delayed.append((i + 4, "norm", 50, p_norm_make(qg, 33)))
                        delayed.append((i + 4, "norm", 50, p_mul_make(qg, 0)))
                        delayed.append((i + 4, "norm", 50, p_mul_make(qg, 1)))
                elif not last_qg:
                    for hh in range(2):
                        pending_av.append(
                            (kt, "norm", 50, p_ul_make(pr, qg, hh))
                        )
                    delayed.append((i + 4, "norm", 50, p_norm_make(qg, 97)))
                    for h2 in range(4):
                        delayed.append(
                            (i + 4, "norm", 50,
                             p_mul_make(qg, h2, unlock_wo=(h2 == 3)))
                        )
                else:
                    # last unit: minimal-latency tail for heads 2,3
                    t0 = tail_head_pieces(qg, 0)
                    t1 = tail_head_pieces(qg, 1)
                    for piece in (t0[0], t1[0], t0[1], t1[1], t0[2], t1[2]):
                        pending_av.append((kt, "norm", *piece))
            pop_pieces()
        # ---- tail: drain everything ----
        for _, kind, cost, fn in delayed:
            avq.append((kind, cost, fn))
        delayed.clear()
        while pending_av or avq or defq:
            if not pop_one_av():
                if not pop_one_def():
                    raise RuntimeError("tail deadlock")


_NC_CACHE = None


def get_nc():
    global _NC_CACHE
    if _NC_CACHE is None:
        _NC_CACHE = build_nc()
    return _NC_CACHE


def make_in_maps(x, Wq, bq, Wk, bk, Wv, bv, Wo, bo):
    import ml_dtypes

    bf16 = ml_dtypes.bfloat16

    def w_arr(W, sl):
        # [D, DG] -> [p, kt*DG]: W[kt*128+p, n] at [p, kt, n]
        return np.ascontiguousarray(
            W[:, sl].reshape(KT_D, P, DG).transpose(1, 0, 2).reshape(P, -1)
        ).astype(bf16)

    in_maps = []
    for c in range(8):
        b, g = c // 4, c % 4
        sl = slice(g * DG, (g + 1) * DG)
        # x[b].T is [D, S]; SBUF wants [p, cb, kt, 512] with row kt*128+p,
        # col cb*512+s
        xt = (
            x[b]
            .T.reshape(KT_D, P, 4, 512)
            .transpose(1, 2, 0, 3)
            .reshape(P, -1)
            .astype(bf16)
        )
        wo_a = np.ascontiguousarray(
            Wo[sl, :].reshape(2, P, D).transpose(1, 0, 2).reshape(P, -1)
        ).astype(bf16)
        in_maps.append(
            {
                "xT": np.ascontiguousarray(xt),
                "wq": w_arr(Wq, sl),
                "wk": w_arr(Wk, sl),
                "wv": w_arr(Wv, sl),
                "wo": wo_a,
                "bq": np.ascontiguousarray(bq[sl].reshape(2, P).T),
                "bk": np.ascontiguousarray(bk[sl].reshape(2, P).T),
                "bv": np.ascontiguousarray(bv[sl].reshape(1, DG)),
            }
        )
    return in_maps


def kernel(x, Wq, bq, Wk, bk, Wv, bv, Wo, bo, _run_kwargs=None):
    from concourse.bass_utils import run_bass_kernel_spmd

    x = np.asarray(x, dtype=np.float32)
    nc = get_nc()
    in_maps = make_in_maps(
        x,
        np.asarray(Wq, np.float32),
        np.asarray(bq, np.float32),
        np.asarray(Wk, np.float32),
        np.asarray(bk, np.float32),
        np.asarray(Wv, np.float32),
        np.asarray(bv, np.float32),
        np.asarray(Wo, np.float32),
        np.asarray(bo, np.float32),
    )
    res = run_bass_kernel_spmd(
        nc, in_maps, core_ids=list(range(8)), **(_run_kwargs or {})
    )
    bo = np.asarray(bo, np.float32)
    outp = np.empty((2, S, D), dtype=np.float32)
    for b in range(2):
        acc = res.results[4 * b]["out"].astype(np.float32)
        for g in range(1, 4):
            acc = acc + res.results[4 * b + g]["out"].astype(np.float32)
        outp[b] = acc + bo[None, :]
    kernel.last_result = res
    return outp


# revision 20
# speedup vs baseline: 1.1502x; 1.0243x over previous
"""Entropy-regularized attention (standard MHA fwd) on 8 trn2 cores.

Sharding: core c -> batch b=c//4, head-group g=c%4 (4 of 16 heads).
Each core computes q/k/v for its 256-wide head-group slice, transposed-
layout attention, then a row-split Wo partial product. Host sums the 4
partials per batch and adds bo (the "all-reduce").

v3 restructure vs v2 (261us -> target ~160us):
- Scores are emitted as ROW-TILED HEAD PAIRS: the two heads of an mt
  group have K=64 stationaries at base partitions 0 and 64, so their
  score matmuls auto-derive tile_position (0,0)/(64,0) and execute
  CONCURRENTLY on the PE (each K=64 tile uses half the array rows).
  This halves the PE time of the scores phase.
- QG=512 with 4 query groups: per pair-unit one [128,2,512] PSUM tile
  holds both heads' scores and ONE Exp activation (N=1024) covers the
  pair, keeping ACT's per-instruction overhead amortized while PSUM
  stays within 8 banks (sc 2x2 + av 2 + ops 2).
- Softmax normalization no longer uses ACT Ln/Exp nor PE broadcast
  matmuls: DVE reciprocal computes 1/l, GPSIMD partition_broadcast
  fans it across 64 partitions, DVE tensor_mul normalizes into avT.
  ACT does nothing but the exp stream; the PE does nothing but matmul.
- ALL non-score PE work (q/k/v projections, av accumulation, Wo
  partials) lives in deferred queues popped under a per-unit cost
  budget (~880ns) between score emissions, so the exp stream is never
  blocked by more than one unit's worth of filler. av matmuls are
  gated on their v-projection pieces to preserve FIFO progress.
- Unit order: pair 01 for all 4 qgs (kt-inner), then pair 23. kT/qT
  mt=1 projections amortize over the first 64 units; Wo partials for
  each qg unlock after its second pair's norms and fill the late
  units, leaving only qg3's Wo (+ final norm chain) in the tail.
"""

import sys

for _p in ("/opt/trn_rl_repo", "/root/.axon_site/_ro/trn_rl_repo"):
    if _p not in sys.path:
        sys.path.insert(0, _p)

import numpy as np

import concourse.bass as bass
import concourse.mybir as mybir
import concourse.tile as tile
from concourse import bacc

P = 128
S = 2048  # sequence length
D = 1024  # hidden
DG = 256  # per-core head-group width (4 heads x 64)
HD = 64
NHL = 4  # heads per core
KT_D = D // P  # 8 contraction tiles for projections
ST = S // P  # 16 sequence tiles
QG = 512  # query-group width
NQG = S // QG  # 4 query groups
NPR = 2  # head pairs per core

F32 = mybir.dt.float32
F32R = mybir.dt.float32r
BF16 = mybir.dt.bfloat16


def build_nc():
    nc = bacc.Bacc(None, target_bir_lowering=False)

    # all inputs pre-arranged on the host into the exact SBUF layouts so
    # every DMA is a contiguous-per-partition blob (large descriptors)
    xT = nc.dram_tensor("xT", [P, 4 * KT_D * 512], BF16, kind="ExternalInput")
    wq = nc.dram_tensor("wq", [P, KT_D * DG], BF16, kind="ExternalInput")
    wk = nc.dram_tensor("wk", [P, KT_D * DG], BF16, kind="ExternalInput")
    wv = nc.dram_tensor("wv", [P, KT_D * DG], BF16, kind="ExternalInput")
    wo = nc.dram_tensor("wo", [P, 2 * D], BF16, kind="ExternalInput")
    bq = nc.dram_tensor("bq", [P, 2], F32, kind="ExternalInput")
    bk = nc.dram_tensor("bk", [P, 2], F32, kind="ExternalInput")
    bv = nc.dram_tensor("bv", [1, DG], F32R, kind="ExternalInput")
    out = nc.dram_tensor("out", [S, D], BF16, kind="ExternalOutput")

    with tile.TileContext(nc) as tc:
        _body(tc, nc, xT, wq, wk, wv, wo, bq, bk, bv, out)

    # Pin Exp/Ln to the one table set holding both: strip them from the
    # competing sets (dict order and size preserved, so act_func_set_id
    # indices stay valid). Without this the table-load pass can bounce
    # between table sets.
    import concourse.bacc as _bacc_mod

    _orig_tables = _bacc_mod.get_activation_tables

    def _pinned_tables(arch):
        t = _orig_tables(arch)
        for name, fns in t.items():
            if name != "natural_log_exp_and_others":
                fns.discard(mybir.ActivationFunctionType.Exp)
                fns.discard(mybir.ActivationFunctionType.Ln)
        return t

    _bacc_mod.get_activation_tables = _pinned_tables
    try:
        nc.compile()
    finally:
        _bacc_mod.get_activation_tables = _orig_tables
    return nc


def _body(tc, nc, xT, wq, wk, wv, wo, bq, bk, bv, out):
    from collections import deque
    from contextlib import ExitStack

    with ExitStack() as ctx:
        ctx.enter_context(
            nc.allow_low_precision(
                reason="bf16 matmul inputs; accumulation is fp32 PSUM"
            )
        )
        persist = ctx.enter_context(tc.tile_pool(name="persist", bufs=1))
        expool = ctx.enter_context(tc.tile_pool(name="expool", bufs=16))
        ulpool = ctx.enter_context(tc.tile_pool(name="ulpool", bufs=12))
        rpool = ctx.enter_context(tc.tile_pool(name="rpool", bufs=4))
        l4pool = ctx.enter_context(tc.tile_pool(name="l4pool", bufs=4))
        r4pool = ctx.enter_context(tc.tile_pool(name="r4pool", bufs=2))
        opool = ctx.enter_context(tc.tile_pool(name="opool", bufs=3))
        # PSUM budget (8 banks): sc 2x[128,2,512]=4, av 2x[65,512]=2,
        # ops 2x[128,512]=2. qkv/v/proj/wo tiles all use the ops slots.
        ps_sc = ctx.enter_context(tc.tile_pool(name="ps_sc", bufs=2, space="PSUM"))
        ps_av = ctx.enter_context(tc.tile_pool(name="ps_av", bufs=2, space="PSUM"))
        ps_o = ctx.enter_context(tc.tile_pool(name="ps_o", bufs=2, space="PSUM"))

        qT_sb = persist.tile([P, 2, S], BF16)
        kT_sb = persist.tile([P, 2, S], BF16)
        v_sb = persist.tile([P, ST, NHL * 65], BF16)  # 65-striped: col 64 = ones
        avT = [
            persist.tile([P, 2, QG], BF16, tag=f"avT{g}", name=f"avT{g}")
            for g in range(NQG)
        ]
        wo_sb = persist.tile([P, 2, D], BF16)
        ones_row = persist.tile([1, P], F32R)
        xT_sb = persist.tile([P, 4, KT_D, 512], BF16)
        wq_sb = persist.tile([P, KT_D, DG], BF16, tag="wq")
        wk_sb = persist.tile([P, KT_D, DG], BF16, tag="wk")
        wv_sb = persist.tile([P, KT_D, DG], BF16, tag="wv")
        bq_sb = persist.tile([P, 2], F32, tag="bq")
        bk_sb = persist.tile([P, 2], F32, tag="bk")
        bv_sb = persist.tile([1, DG], F32R, tag="bv")

        # DMAs ordered to match first-use: wk then the first xT column
        # block gate the first projection chain.
        xTr = xT.rearrange("p (cb kt s) -> p cb kt s", kt=KT_D, s=512)
        nc.sync.dma_start(wk_sb[:], wk.rearrange("p (kt n) -> p kt n", n=DG))
        nc.sync.dma_start(xT_sb[:, 0, 0:4], xTr[:, 0, 0:4])
        nc.sync.dma_start(bk_sb[:], bk[:])
        nc.sync.dma_start(wq_sb[:], wq.rearrange("p (kt n) -> p kt n", n=DG))
        nc.sync.dma_start(xT_sb[:, 0, 4:8], xTr[:, 0, 4:8])
        nc.sync.dma_start(bq_sb[:], bq[:])
        nc.sync.dma_start(wv_sb[:], wv.rearrange("p (kt n) -> p kt n", n=DG))
        nc.sync.dma_start(bv_sb[:], bv[:])
        nc.sync.dma_start(xT_sb[:, 1], xTr[:, 1])
        nc.sync.dma_start(xT_sb[:, 2], xTr[:, 2])
        nc.sync.dma_start(xT_sb[:, 3], xTr[:, 3])
        nc.sync.dma_start(wo_sb[:], wo.rearrange("p (kt n) -> p kt n", n=D))

        # memset can't emit float32r; stage fp32 ones and copy-cast
        ones_f32 = persist.tile([P, P], F32)
        nc.vector.memset(ones_f32[:], 1.0)
        nc.vector.tensor_copy(ones_row[:], ones_f32[0:1, :])
        ones_all = persist.tile([P, P], BF16)
        nc.vector.tensor_copy(ones_all[:], ones_f32[:])
        nc.vector.tensor_copy(
            v_sb.rearrange("p st (h w) -> p st h w", w=65)[:, :, :, 64],
            ones_f32[:, 0:64].rearrange("p (st h) -> p st h", h=4),
        )

        # ---- projection building blocks ----
        def qk_half(ps, wsb, mt, nq, kts):
            for kt in kts:
                nc.tensor.matmul(
                    ps[:, 0:512],
                    wsb[:, kt, mt * P : (mt + 1) * P],
                    xT_sb[:, nq, kt, :],
                    start=(kt == 0),
                    stop=(kt == KT_D - 1),
                )

        def qk_evict(ps, bsb, dest, mt, nq):
            nc.vector.tensor_scalar_add(
                dest[:, mt, nq * 512 : (nq + 1) * 512],
                ps[:, 0:512],
                bsb[:, mt : mt + 1],
            )

        # emission-order readiness flags: a score matmul may only be
        # EMITTED after the projection chain writing its qT/kT region has
        # been emitted (the Tile framework tracks writer->reader deps in
        # program order; a later-emitted writer would be a race).
        qk_ready = {}  # ("q"/"k", mt, nq) -> True

        def qk_full(wsb, bsb, dest, mt, nq, key):
            ps = ps_o.tile([P, 512], F32, tag="ops", name="qkps")
            qk_half(ps, wsb, mt, nq, range(0, KT_D))
            qk_evict(ps, bsb, dest, mt, nq)
            qk_ready[(key, mt, nq)] = True

        def qk_pieces(wsb, bsb, dest, mt, nq, key, pieces=4):
            # split the 8-matmul K-chain into `pieces` closures; the last
            # one carries the bias eviction. cost ~= (8/pieces)*213ns
            box = []
            step = KT_D // pieces

            def make(pi):
                def run():
                    if pi == 0:
                        box.append(
                            ps_o.tile([P, 512], F32, tag="ops", name="qkps")
                        )
                    ps = box[-1]
                    qk_half(ps, wsb, mt, nq, range(pi * step, (pi + 1) * step))
                    if pi == pieces - 1:
                        qk_evict(ps, bsb, dest, mt, nq)
                        box.pop()
                        qk_ready[(key, mt, nq)] = True

                return run

            return [(step * 215, make(pi)) for pi in range(pieces)]

        v_done = [False] * ST

        def v_group_half(st, half):
            # half 0: kts 0-3; half 1: kts 4-7 + bias + evict
            def run():
                if half == 0:
                    v_group_half.box[st] = ps_o.tile(
                        [P, 512], F32, tag="ops", name="vps"
                    )
                    ps = v_group_half.box[st]
                    for kt in range(0, 4):
                        nc.tensor.matmul(
                            ps[:, 0:DG],
                            xT_sb[:, st // 4, kt, (st % 4) * P : (st % 4 + 1) * P],
                            wv_sb[:, kt, :],
                            start=(kt == 0),
                            stop=False,
                        )
                else:
                    ps = v_group_half.box.pop(st)
                    for kt in range(4, KT_D):
                        nc.tensor.matmul(
                            ps[:, 0:DG],
                            xT_sb[:, st // 4, kt, (st % 4) * P : (st % 4 + 1) * P],
                            wv_sb[:, kt, :],
                            start=False,
                            stop=False,
                        )
                    nc.tensor.matmul(
                        ps[:, 0:DG],
                        ones_row[0:1, 0:P],
                        bv_sb[0:1, :],
                        start=False,
                        stop=True,
                    )
                    nc.vector.tensor_copy(
                        v_sb.rearrange("p st (h w) -> p st h w", w=65)[
                            :, st, :, 0:64
                        ],
                        ps[:, 0:DG].rearrange("p (h w) -> p h w", w=64),
                    )
                    v_done[st] = True

            return run

        v_group_half.box = {}

        def v_group_full(st):
            v_group_half(st, 0)()
            v_group_half(st, 1)()

        # ---- deferred machinery ----
        # The PE engine queue is strict FIFO: an emitted matmul that waits
        # on a semaphore blocks everything emitted after it. So av pieces
        # (which wait on their exp) are RELEASE-DELAYED ~3 units: by the
        # time they enter the queue their exp has long finished, and the
        # next units' score matmuls are never stuck behind them. `delayed`
        # is strictly FIFO (entries release in order once their
        # release-iter arrives and their v-group gate is satisfied).
        # avq: released attention follow-up (av/norms), popped first.
        # defq: projection / Wo filler under a per-unit cost budget.
        avq = deque()
        defq = deque()
        delayed = deque()  # (release_iter, gate_st_or_None, kind, cost, fn)

        BUDGET = 880
        n_av_emitted = [0]  # count of av matmul pieces emitted (2 per unit)

        def release_delayed(i):
            while delayed and delayed[0][0] <= i:
                _, gate, kind, cost, fn = delayed[0]
                if gate is not None and not v_done[gate]:
                    break
                delayed.popleft()
                avq.append((kind, cost, fn))

        def pop_one_av():
            if avq:
                kind, _, fn = avq.popleft()
                fn()
                if kind == "av":
                    n_av_emitted[0] += 1
                return True
            return False

        def pop_one_def():
            if defq:
                _, fn = defq.popleft()
                fn()
                return True
            return False

        def pop_pieces(i):
            spent = 0
            release_delayed(i)
            while avq and spent < BUDGET:
                kind, cost, fn = avq.popleft()
                fn()
                if kind == "av":
                    n_av_emitted[0] += 1
                spent += cost
            while defq and spent < BUDGET:
                cost, fn = defq.popleft()
                fn()
                spent += cost

        def ensure_qk(key, mt, nq):
            # force-pop until the projection chain for this region has run
            while not qk_ready.get((key, mt, nq)):
                if not pop_one_def():
                    raise RuntimeError(f"deadlock: {key} mt{mt} nq{nq}")

        def ensure_av_through(unit_idx):
            # all av pieces of units <= unit_idx emitted (ex-slot reuse)
            while n_av_emitted[0] < 2 * (unit_idx + 1):
                if pop_one_av():
                    continue
                if delayed:
                    _, gate, kind, cost, fn = delayed[0]
                    if gate is None or v_done[gate]:
                        delayed.popleft()
                        avq.append((kind, cost, fn))
                        continue
                if not pop_one_def():
                    raise RuntimeError("deadlock: av drain")

        # ---- attention phase ----
        EXBUFS = 16  # must match expool bufs

        def sc_exp(pr, qg, kt):
            ensure_qk("k", pr, kt // 4)
            ensure_qk("q", pr, qg)
            mt, q0 = pr, qg * QG
            sc = ps_sc.tile([P, 2, 512], F32, tag="sc", name="sc")
            for hh in range(2):
                po = hh * 64
                nc.tensor.matmul(
                    sc[:, hh, :],
                    kT_sb[po : po + 64, mt, kt * P : (kt + 1) * P],
                    qT_sb[po : po + 64, mt, q0 : q0 + 512],
                    start=True,
                    stop=True,
                )
            ex = expool.tile([P, 2, 512], BF16, tag="ex", name="ex")
            nc.scalar.activation(
                ex[:], sc[:], mybir.ActivationFunctionType.Exp, scale=0.125
            )
            return ex

        avps = {}  # hh -> live av psum tile for current (pr, qg)

        def av_piece(pr, qg, kt, hh, ex):
            h = 2 * pr + hh

            def run():
                if kt == 0:
                    avps[hh] = ps_av.tile([65, 512], F32, tag="av", name="av")
                nc.tensor.matmul(
                    avps[hh][0:65, :],
                    v_sb[:, kt, h * 65 : h * 65 + 65],
                    ex[:, hh, :],
                    start=(kt == 0),
                    stop=(kt == ST - 1),
                )

            return (215, run)

        # Softmax normalization: per (pr,qg,head) the av PSUM is copied to
        # SBUF (ul) and its denominator row gathered into l4[qg] at
        # partition 32h. Once a qg's heads are all gathered, ONE batched
        # Ln + ONE Exp(-x) on ACT produce 1/l ([97,512] costs the same as
        # [1,512]); GPSIMD broadcasts each head's row across 64 partitions
        # and DVE multiplies into avT. The Ln/Exp+muls are DELAYED ~4
        # units so they land in ACT's FIFO behind already-ready exps
        # (emitting them immediately stalls the exp stream on the av->ul->
        # gather dependency chain). The LAST qg's second pair instead runs
        # per-head Ln/Exp straight from PSUM with a PE-matmul broadcast,
        # minimizing the serial tail after the final exp.
        uls = {}
        l4s = {}
        r4box = {}

        def p_ul_make(pr, qg, hh, skip_l=False):
            h = 2 * pr + hh

            def p_ul():
                av = avps.pop(hh)
                if qg not in l4s:
                    l4s[qg] = l4pool.tile([97, 512], F32, tag="l4", name="l4")
                    nc.gpsimd.memset(l4s[qg][:], 1.0)
                ul = ulpool.tile([65, 512], F32, tag="ul", name="ul")
                nc.vector.tensor_copy(ul[:], av[0:65, :])
                if not skip_l:
                    nc.vector.tensor_copy(
                        l4s[qg][32 * h : 32 * h + 1, :], av[64:65, :]
                    )
                uls[(qg, h)] = ul

            return p_ul

        def p_norm_make(qg, hi):
            def p_norm():
                # 1/l = exp(-ln(l)) for heads 0..hi/32 in one Ln + one Exp
                l4 = l4s.pop(qg)
                ls = r4pool.tile([97, 512], F32, tag="ls", name="ls")
                nc.scalar.activation(
                    ls[0:hi], l4[0:hi], mybir.ActivationFunctionType.Ln
                )
                r4 = r4pool.tile([97, 512], F32, tag="r4", name="r4")
                nc.scalar.activation(
                    r4[0:hi], ls[0:hi],
                    mybir.ActivationFunctionType.Exp, scale=-1.0,
                )
                r4box[qg] = r4

            return p_norm

        def p_mul_make(qg, h2, unlock_wo=False):
            def p_mul():
                mt2, po2 = h2 // 2, (h2 % 2) * 64
                ul = uls.pop((qg, h2))
                # partition_broadcast reads physical partition 0: stage
                # this head's reciprocal row to a base-0 tile first
                r1 = rpool.tile([1, 512], F32, tag="r1", name="r1")
                nc.vector.tensor_copy(
                    r1[:], r4box[qg][32 * h2 : 32 * h2 + 1, :]
                )
                rbb = rpool.tile([64, 512], F32, tag="rbb", name="rbb")
                nc.gpsimd.partition_broadcast(rbb[:], r1[:], channels=64)
                nc.vector.tensor_mul(
                    out=avT[qg][po2 : po2 + 64, mt2, :],
                    in0=ul[0:64, :],
                    in1=rbb[:],
                )
                if unlock_wo:
                    defq.extend(wo_pieces(qg))

            return p_mul

        def tail_head_pieces(qg, hh):
            # last qg, second pair: Ln/Exp straight from the av PSUM, PE
            # rb-matmul broadcast (PE is idle in the tail), DVE multiply.
            h = 2 + hh
            box = {}

            def p_lnexp():
                av = avps[hh]  # keep psum live; popped in p_ul below
                lnr = rpool.tile([1, 512], F32, tag="lnr", name="lnr")
                nc.scalar.activation(
                    lnr[:], av[64:65, :], mybir.ActivationFunctionType.Ln
                )
                r1 = rpool.tile([1, 512], BF16, tag="r1b", name="r1b")
                nc.scalar.activation(
                    r1[:], lnr[:], mybir.ActivationFunctionType.Exp,
                    scale=-1.0,
                )
                box["r1"] = r1

            def p_ul():
                av = avps.pop(hh)
                ul = ulpool.tile([65, 512], F32, tag="ul", name="ul")
                nc.vector.tensor_copy(ul[0:64, :], av[0:64, :])
                box["ul"] = ul

            def p_rbmul():
                rb = ps_o.tile([P, 512], F32, tag="ops", name="rb")
                nc.tensor.matmul(
                    rb[0:64, :],
                    ones_all[0:1, 0:64],
                    box["r1"][:],
                    start=True,
                    stop=True,
                )
                nc.vector.tensor_mul(
                    out=avT[qg][hh * 64 : hh * 64 + 64, 1, :],
                    in0=box["ul"][0:64, :],
                    in1=rb[0:64, :],
                )
                if hh == 1:
                    defq.extend(wo_pieces(qg))

            return [(50, p_lnexp), (50, p_ul), (250, p_rbmul)]

        ot_box = {}

        def wo_pieces(qg):
            # per (sti, nd): 2 matmuls (kt2 accumulation) + eviction; the
            # ops-pool slot is alloc'd and freed within one piece so the
            # bufs=2 rotation can interleave with qk/v pieces.
            pieces = []

            def make(sti, nd):
                def run():
                    st = qg * (QG // P) + sti
                    if nd == 0:
                        ot_box[sti] = opool.tile([P, D], BF16, tag="ot", name="ot")
                    ot = ot_box[sti]
                    pp = ps_o.tile([P, 512], F32, tag="ops", name="pp")
                    for kt2 in range(2):
                        nc.tensor.matmul(
                            pp[:],
                            avT[qg][:, kt2, sti * P : (sti + 1) * P],
                            wo_sb[:, kt2, nd * 512 : (nd + 1) * 512],
                            start=(kt2 == 0),
                            stop=(kt2 == 1),
                        )
                    nc.vector.tensor_copy(ot[:, nd * 512 : (nd + 1) * 512], pp[:])
                    if nd == 1:
                        del ot_box[sti]
                        nc.sync.dma_start(out[st * P : (st + 1) * P, :], ot[:])

                return run

            for sti in range(QG // P):
                for nd in range(2):
                    pieces.append((460, make(sti, nd)))
            return pieces

        # ---- upfront phase: only what the FIRST sc/exp needs (k+q mt0
        # nq0); v groups are deferred (av lags behind the exp stream) ----
        qk_full(wk_sb, bk_sb, kT_sb, 0, 0, "k")
        qk_full(wq_sb, bq_sb, qT_sb, 0, 0, "q")

        # ---- static filler: ordered by need-by unit ----
        # kT mt0 fully by unit ~12 (sc consumes kt blocks 4/8/12 at units
        # 4/8/12); q(mt0,nq1..3) by units 16/32/48; v(st) before av(st)
        # emission (forced by ensure_av_through); mt1 chains by unit 64.
        for st in (0, 1):
            defq.append((500, v_group_half(st, 0)))
            defq.append((560, v_group_half(st, 1)))
        defq.extend(qk_pieces(wk_sb, bk_sb, kT_sb, 0, 1, "k"))
        defq.extend(qk_pieces(wk_sb, bk_sb, kT_sb, 0, 2, "k"))
        defq.extend(qk_pieces(wk_sb, bk_sb, kT_sb, 0, 3, "k"))
        defq.extend(qk_pieces(wq_sb, bq_sb, qT_sb, 0, 1, "q"))
        for st in (2, 3, 4, 5):
            defq.append((500, v_group_half(st, 0)))
            defq.append((560, v_group_half(st, 1)))
        defq.extend(qk_pieces(wq_sb, bq_sb, qT_sb, 0, 2, "q"))
        for st in (6, 7, 8, 9):
            defq.append((500, v_group_half(st, 0)))
            defq.append((560, v_group_half(st, 1)))
        defq.extend(qk_pieces(wq_sb, bq_sb, qT_sb, 0, 3, "q"))
        for st in (10, 11, 12, 13, 14, 15):
            defq.append((500, v_group_half(st, 0)))
            defq.append((560, v_group_half(st, 1)))
        for nq in range(4):
            defq.extend(qk_pieces(wk_sb, bk_sb, kT_sb, 1, nq, "k"))
            defq.extend(qk_pieces(wq_sb, bq_sb, qT_sb, 1, nq, "q"))

        # ---- the unit loop ----
        units = [
            (pr, qg, kt)
            for pr in range(NPR)
            for qg in range(NQG)
            for kt in range(ST)
        ]
        ex_tiles = {0: sc_exp(*units[0])}
        for i, (pr, qg, kt) in enumerate(units):
            if i + 1 < len(units):
                ensure_av_through(i + 1 - EXBUFS)
                ex_tiles[i + 1] = sc_exp(*units[i + 1])
            ex = ex_tiles.pop(i)
            for hh in range(2):
                delayed.append((i + 3, kt, "av", *av_piece(pr, qg, kt, hh, ex)))
            if kt == ST - 1:
                last_qg = qg == NQG - 1
                if pr == 0:
                    for hh in range(2):
                        delayed.append(
                            (i + 4, None, "norm", 50, p_ul_make(pr, qg, hh))
                        )
                    if last_qg:
                        # qg3 pair-0 heads: batched 2-head norm
                        delayed.append(
                            (i + 5, None, "norm", 50, p_norm_make(qg, 33))
                        )
                        delayed.append(
                            (i + 5, None, "norm", 50, p_mul_make(qg, 0))
                        )
                        delayed.append(
                            (i + 5, None, "norm", 50, p_mul_make(qg, 1))
                        )
                elif not last_qg:
                    for hh in range(2):
                        delayed.append(
                            (i + 4, None, "norm", 50, p_ul_make(pr, qg, hh))
                        )
                    delayed.append(
                        (i + 5, None, "norm", 50, p_norm_make(qg, 97))
                    )
                    for h2 in range(4):
                        delayed.append(
                            (i + 5, None, "norm", 50,
                             p_mul_make(qg, h2, unlock_wo=(h2 == 3)))
                        )
                else:
                    # last unit: minimal-latency tail for heads 2,3
                    t0 = tail_head_pieces(qg, 0)
                    t1 = tail_head_pieces(qg, 1)
                    for piece in (t0[0], t1[0], t0[1], t1[1], t0[2], t1[2]):
                        delayed.append((i, None, "norm", *piece))
            pop_pieces(i)
        # ---- tail: drain everything ----
        while delayed or avq or defq:
            release_delayed(10 ** 9)
            if pop_one_av():
                continue
            if not pop_one_def():
                if delayed:
                    raise RuntimeError("tail deadlock")


_NC_CACHE = None


def get_nc():
    global _NC_CACHE
    if _NC_CACHE is None:
        _NC_CACHE = build_nc()
    return _NC_CACHE


def make_in_maps(x, Wq, bq, Wk, bk, Wv, bv, Wo, bo):
    import ml_dtypes

    bf16 = ml_dtypes.bfloat16

    def w_arr(W, sl):
        # [D, DG] -> [p, kt*DG]: W[kt*128+p, n] at [p, kt, n]
        return np.ascontiguousarray(
            W[:, sl].reshape(KT_D, P, DG).transpose(1, 0, 2).reshape(P, -1)
        ).astype(bf16)

    in_maps = []
    for c in range(8):
        b, g = c // 4, c % 4
        sl = slice(g * DG, (g + 1) * DG)
        # x[b].T is [D, S]; SBUF wants [p, cb, kt, 512] with row kt*128+p,
        # col cb*512+s
        xt = (
            x[b]
            .T.reshape(KT_D, P, 4, 512)
            .transpose(1, 2, 0, 3)
            .reshape(P, -1)
            .astype(bf16)
        )
        wo_a = np.ascontiguousarray(
            Wo[sl, :].reshape(2, P, D).transpose(1, 0, 2).reshape(P, -1)
        ).astype(bf16)
        in_maps.append(
            {
                "xT": np.ascontiguousarray(xt),
                "wq": w_arr(Wq, sl),
                "wk": w_arr(Wk, sl),
                "wv": w_arr(Wv, sl),
                "wo": wo_a,
                "bq": np.ascontiguousarray(bq[sl].reshape(2, P).T),
                "bk": np.ascontiguousarray(bk[sl].reshape(2, P).T),
                "bv": np.ascontiguousarray(bv[sl].reshape(1, DG)),
            }
        )
    return in_maps


def kernel(x, Wq, bq, Wk, bk, Wv, bv, Wo, bo, _run_kwargs=None):
    from concourse.bass_utils import run_bass_kernel_spmd

    x = np.asarray(x, dtype=np.float32)
    nc = get_nc()
    in_maps = make_in_maps(
        x,
        np.asarray(Wq, np.float32),
        np.asarray(bq, np.float32),
        np.asarray(Wk, np.float32),
        np.asarray(bk, np.float32),
        np.asarray(Wv, np.float32),
        np.asarray(bv, np.float32),
        np.asarray(Wo, np.float32),
        np.asarray(bo, np.float32),
    )
    res = run_bass_kernel_spmd(
        nc, in_maps, core_ids=list(range(8)), **(_run_kwargs or {})
    )
    bo = np.asarray(bo, np.float32)
    outp = np.empty((2, S, D), dtype=np.float32)
    for b in range(2):
        acc = res.results[4 * b]["out"].astype(np.float32)
        for g in range(1, 4):
            acc = acc + res.results[4 * b + g]["out"].astype(np.float32)
        outp[b] = acc + bo[None, :]
    kernel.last_result = res
    return outp
